# revision 1
# baseline (speedup 1.0000x reference)
"""Causal single-head attention (B=4, T=4096, C=1024, H=64) on 8 TRN2 cores.

Sharding: 2 cores per batch element, causal-balanced interleaved query
blocks of 512: half 0 owns query blocks {0, 2, 4, 6}, half 1 owns
{1, 3, 5, 7}.  Each core projects Q^T, K^T, V for all T columns from a
host-transposed X^T slice (X^T is required because TensorE contracts
over the partition dim), then runs a transposed-flash attention loop
(S^T layout) so no on-device transposes of big tensors are needed.

All 8 cores run ONE identical SPMD program.  Per-core asymmetry is
carried in input DATA only:
  - structural slots with k-tile counts [8, 16, 24, 32] (>= both halves'
    causal prefixes per slot; half 0 wastes 4 masked tiles per slot),
  - qoff: per-slot column offset of the slot's queries inside the global
    Q^T buffer, consumed via a register-offset (dynamic) access pattern,
  - tau thresholds [128, 32]: mask[kp, qf] = (qf >= tau), applied to the
    last 8 k-tiles of every slot as one fused DVE op against an iota row.

Matmuls run as float32r (full PE rate at free dim >= 256).  Softmax
skips the running-max pass (scores ~ N(0,1) after the H^-0.5 scale, so
exp is safe in fp32); the denominator comes from a ones-column appended
to V, and the division is folded in after a small PE transpose of O^T.
"""

import os

import numpy as np

import concourse.bacc as bacc
import concourse.mybir as mybir
import concourse.tile as tile
from concourse.bass_utils import run_bass_kernel_spmd
from concourse.masks import make_identity

B, T, C, H = 4, 4096, 1024, 64
NCORES = 8
QB = 512  # query block (matmul free dim)
KT = 128  # key tile (S^T partition dim)
CCH = C // 128  # contraction chunks
SLOT_TILES = [8, 16, 24, 32]  # structural k-tiles per slot
MASKN = 4  # masked tail tile-PAIRS per slot
NQ = 4 * QB  # queries per core (2048)
F32 = mybir.dt.float32
F32R = mybir.dt.float32r
F16 = mybir.dt.float16
I32 = mybir.dt.int32
XD = F16 if os.environ.get("KERNEL_XDTYPE", "f16") == "f16" else F32R
XNP = np.float16 if os.environ.get("KERNEL_XDTYPE", "f16") == "f16" else np.float32

# slot -> actual query-block base, per half
SLOT_QBASE = {
    0: [0, 1024, 2048, 3072],  # prefix tiles 4, 12, 20, 28
    1: [512, 1536, 2560, 3584],  # prefix tiles 8, 16, 24, 32
}

_PROGRAM = None


def _build_program():
    nc = bacc.Bacc(None, target_bir_lowering=False, debug=False)

    xt = nc.dram_tensor("xt", [C, T], XD, kind="ExternalInput")
    wqk = nc.dram_tensor("wqk", [C, 128], XD, kind="ExternalInput")
    wv = nc.dram_tensor("wv", [C, 64], XD, kind="ExternalInput")
    tau = nc.dram_tensor("tau", [128, 4 * MASKN], F32, kind="ExternalInput")
    sel = nc.dram_tensor("sel", [64, 2], F32, kind="ExternalInput")
    o = nc.dram_tensor("o", [NQ, H], F32, kind="ExternalOutput")

    xt_r = xt.rearrange("(n p) t -> p n t", p=128)  # [128, 8, T]
    wqk_r = wqk.rearrange("(n p) m -> p n m", p=128)  # [128, 8, 128]
    wv_r = wv.rearrange("(n p) m -> p n m", p=128)  # [128, 8, 64]

    def mm(out_ap, lhsT, rhs, start, stop):
        nc.tensor.matmul(out_ap, lhsT, rhs, start=start, stop=stop)

    with tile.TileContext(nc) as tc:
        with (
            tc.tile_pool(name="const", bufs=1) as const_pool,
            tc.tile_pool(name="big", bufs=1) as big_pool,
            tc.tile_pool(name="xin", bufs=4) as xin_pool,
            tc.tile_pool(name="stage", bufs=3) as stage_pool,
            tc.tile_pool(name="p", bufs=3) as p_pool,
            tc.tile_pool(name="outp", bufs=3) as out_pool,
            tc.tile_pool(name="ps_st", bufs=2, space="PSUM") as ps_st,
            tc.tile_pool(name="ps_proj", bufs=2, space="PSUM") as ps_proj,
            tc.tile_pool(name="ps_o", bufs=1, space="PSUM") as ps_o,
            tc.tile_pool(name="ps_tr", bufs=1, space="PSUM") as ps_tr,
        ):
            # ---- constants ----
            ident = const_pool.tile([128, 65], F32)
            make_identity(nc, ident[0:65, 0:65])
            ident_h = const_pool.tile([64, 64], XD)
            make_identity(nc, ident_h[:])
            wqk_s = const_pool.tile([128, CCH, 128], XD)
            nc.sync.dma_start(wqk_s[:], wqk_r)
            wv_s = const_pool.tile([128, CCH, 64], XD)
            nc.sync.dma_start(wv_s[:], wv_r)

            iota_i = const_pool.tile([128, 2, QB], I32)
            nc.gpsimd.iota(
                iota_i[:], pattern=[[-KT, 2], [1, QB]], base=0, channel_multiplier=0
            )
            iota_f = const_pool.tile([128, 2, QB], XD)
            nc.vector.tensor_copy(iota_f[:], iota_i[:])
            tau_s = const_pool.tile([128, 4 * MASKN], F32)
            sel_s = const_pool.tile([64, 2], F32)

            # ---- persistent activations ----
            # K^T folded for row-packed S^T: global k-tile 2i+h lives at
            # rows [64h:64h+64), cols [i*128:(i+1)*128)
            kT_s = big_pool.tile([128, T // 2], XD)
            qcand_s = big_pool.tile([64, 4, 2 * QB], XD)  # per-slot q candidates
            v_s = big_pool.tile([128, T // 128, 65], XD)  # V rows + ones col
            ones_sb = const_pool.tile([128, T // 128, 1], F32)
            nc.vector.memset(ones_sb[:], 1.0)
            nc.vector.tensor_copy(v_s[:, :, 64:65], ones_sb[:])


            def project_block(tb):
                sl = slice(tb * QB, (tb + 1) * QB)
                xt_t = xin_pool.tile([128, CCH, QB], XD, tag="xin")
                for ck in range(0, CCH, 2):
                    nc.sync.dma_start(
                        xt_t[:, ck : ck + 2], xt_r[:, ck : ck + 2, sl]
                    )
                qk_ps = ps_proj.tile([128, QB], F32, tag="ps_proj")
                for ci in range(CCH):
                    mm(qk_ps[:], wqk_s[:, ci], xt_t[:, ci], ci == 0, ci == CCH - 1)
                nc.vector.tensor_copy(
                    qcand_s[:, tb // 2, (tb % 2) * QB : (tb % 2) * QB + QB],
                    qk_ps[0:64, :],
                )
                kv = qk_ps[64:128, :].rearrange("p (n c) -> p n c", c=KT)
                kf = kT_s[:, tb * 2 * KT : (tb + 1) * 2 * KT].rearrange(
                    "p (n c) -> p n c", c=KT
                )
                nc.scalar.copy(kf[0:64], kv[:, 0::2])
                nc.vector.tensor_copy(kf[64:128], kv[:, 1::2])
                vt_ps = ps_proj.tile([128, QB], F32, tag="ps_proj")
                for ci in range(CCH):
                    mm(vt_ps[0:64, :], wv_s[:, ci], xt_t[:, ci], ci == 0, ci == CCH - 1)
                vt_sb = stage_pool.tile([64, QB], XD, tag="vt")
                nc.vector.tensor_copy(vt_sb[:], vt_ps[0:64, :])
                for i in range(QB // 128):
                    v_ps = ps_tr.tile([128, 64], XD, tag="ps_tr")
                    nc.tensor.transpose(
                        v_ps[:], vt_sb[:, i * 128 : (i + 1) * 128], ident_h[:]
                    )
                    nc.vector.tensor_copy(v_s[:, tb * 4 + i, 0:64], v_ps[:])

            def attend_slot(j):
                ntiles = SLOT_TILES[j]
                # select our half's candidate q-block: q = even*(1-h) + odd*h
                # (duplicated on partitions 0:64 and 64:128 for row-packing)
                q_sb = stage_pool.tile([128, QB], XD, tag="qslot")
                nc.vector.tensor_scalar_mul(
                    q_sb[0:64, :], qcand_s[:, j, 0:QB], sel_s[:, 0:1]
                )
                nc.vector.scalar_tensor_tensor(
                    q_sb[0:64, :],
                    qcand_s[:, j, QB : 2 * QB],
                    sel_s[:, 1:2],
                    q_sb[0:64, :],
                    mybir.AluOpType.mult,
                    mybir.AluOpType.add,
                )
                nc.scalar.copy(q_sb[64:128, :], q_sb[0:64, :])
                o_ps = ps_o.tile([65, QB], F32, tag="ps_o")
                for tp in range(ntiles // 2):  # paired k-tiles share one exp
                    st_ps = ps_st.tile([128, 2, QB], F32, tag="ps_st")
                    for h in range(2):
                        nc.tensor.matmul(
                            st_ps[:, h],
                            kT_s[64 * h : 64 * h + 64, tp * KT : (tp + 1) * KT],
                            q_sb[64 * h : 64 * h + 64, :],
                            start=True,
                            stop=True,
                            tile_position=(64 * h, 0),
                        )
                    p_sb = p_pool.tile([128, 2, QB], XD, tag="p")
                    nc.scalar.activation(
                        p_sb[:],
                        st_ps[:],
                        mybir.ActivationFunctionType.Exp,
                        scale=float(H) ** -0.5,
                    )
                    relp = tp - (ntiles // 2 - MASKN)
                    if relp >= 0:
                        nc.vector.scalar_tensor_tensor(
                            p_sb[:],
                            iota_f[:],
                            tau_s[:, j * MASKN + relp : j * MASKN + relp + 1],
                            p_sb[:],
                            mybir.AluOpType.is_ge,
                            mybir.AluOpType.mult,
                        )
                    for h in range(2):
                        ti = 2 * tp + h
                        mm(o_ps[:], v_s[:, ti], p_sb[:, h], ti == 0, ti == ntiles - 1)

                # ---- normalize + store ----
                ot_sb = stage_pool.tile([65, QB], F32, tag="ot")
                nc.vector.tensor_copy(ot_sb[:], o_ps[:])
                for i in range(QB // 128):
                    tr_ps = ps_tr.tile([128, 65], F32, tag="ps_tr")
                    nc.tensor.transpose(
                        tr_ps[:], ot_sb[:, i * 128 : (i + 1) * 128], ident[0:65, 0:65]
                    )
                    recip = out_pool.tile([128, 1], F32, tag="recip")
                    nc.vector.reciprocal(recip[:], tr_ps[:, 64:65])
                    o_sb = out_pool.tile([128, H], F32, tag="o_sb")
                    nc.vector.tensor_scalar_mul(o_sb[:], tr_ps[:, 0:64], recip[:])
                    nc.sync.dma_start(
                        o[j * QB + i * 128 : j * QB + (i + 1) * 128, :], o_sb[:]
                    )

            # interleave: slot j's keys/queries are ready after t-block 2j+1
            nc.sync.dma_start(tau_s[:], tau[:])
            nc.sync.dma_start(sel_s[:], sel[:])
            probe = os.environ.get("KERNEL_PROBE", "")
            for j in range(4):
                if probe != "attn_only":
                    project_block(2 * j)
                    project_block(2 * j + 1)
                if probe != "proj_only":
                    attend_slot(j)

    nc.compile()
    return nc


def _tau_table(half: int) -> np.ndarray:
    """tau[kp, j*MASKN + relp]: threshold of the FIRST tile of pair relp in
    slot j's masked tail; half h of the pair is masked as
    (qf - 128*h >= tau).  Valid range must clip to [-128, 640] so that a
    pair whose both tiles are fully valid / fully masked works for both
    halves."""
    t = np.zeros((128, 4 * MASKN), dtype=np.float32)
    kp = np.arange(128)
    for j in range(4):
        qb = SLOT_QBASE[half][j]
        for relp in range(MASKN):
            ti = SLOT_TILES[j] - 2 * MASKN + 2 * relp
            key_g = ti * KT + kp
            t[:, j * MASKN + relp] = np.clip(key_g - qb, -KT, QB + KT + 1)
    return t


def kernel(X, Wq, Wk, Wv):
    global _PROGRAM
    X = np.asarray(X, dtype=np.float32)
    Wq = np.asarray(Wq, dtype=np.float32)
    Wk = np.asarray(Wk, dtype=np.float32)
    Wv = np.asarray(Wv, dtype=np.float32)

    if _PROGRAM is None:
        _PROGRAM = _build_program()
    nc = _PROGRAM

    wqk = np.ascontiguousarray(np.concatenate([Wq, Wk], axis=1))

    in_maps = []
    for core in range(NCORES):
        b, half = core // 2, core % 2
        xt = np.ascontiguousarray(X[b].T).astype(XNP)  # [C, T]
        in_maps.append(
            {
                "xt": xt,
                "wqk": wqk.astype(XNP),
                "wv": Wv.astype(XNP),
                "tau": _tau_table(half),
                "sel": np.ascontiguousarray(
                    np.broadcast_to(
                        np.asarray([1.0 - half, float(half)], np.float32), (64, 2)
                    )
                ),
            }
        )

    trace = bool(os.environ.get("KERNEL_TRACE"))
    if trace:
        try:
            from antenv.axon_hooks import get_axon_ntff_profile_hook  # noqa: F401
        except ImportError:
            print("KERNEL_TRACE requested but axon NTFF hook unavailable; running untraced")
            trace = False
    kwargs = {}
    if trace:
        kwargs = dict(
            trace=True,
            trace_cores=[
                int(c) for c in os.environ.get("KERNEL_TRACE_CORES", "0").split(",")
            ],
        )
    res = run_bass_kernel_spmd(nc, in_maps, core_ids=list(range(NCORES)), **kwargs)
    if trace:
        print(f"HW exec time: {res.exec_time_ns} ns")
        print(f"mean exec time: {res.mean_exec_time_ns} ns")
        kernel.last_results = res

    out = np.empty((B, T, H), dtype=np.float32)
    for core in range(NCORES):
        b, half = core // 2, core % 2
        oc = res.results[core]["o"]
        for j, qb in enumerate(SLOT_QBASE[half]):
            out[b, qb : qb + QB] = oc[j * QB : (j + 1) * QB]
    return out



# revision 15
# speedup vs baseline: 1.4353x; 1.4353x over previous
"""Causal single-head attention (B=4, T=4096, C=1024, H=64) on 8 TRN2 cores.

Sharding: 2 cores per batch element, causal-balanced interleaved query
blocks of 256: half 0 owns global 256-blocks {0,3,4,7,8,11,12,15}, half 1
owns {1,2,5,6,9,10,13,14}.  With this ownership, a core's slot j (its
j-th owned block, ascending) attends exactly j+1 own blocks and
structurally j+1 partner blocks (one of which may be a fully-masked
filler) — the union is always the causal prefix, and the structural
shape is identical on every core (72 of the ideal 68 [256k x 256q]
tiles vs 80 for a 512-wide split).

The host PERMUTES each core's X^T columns into [own blocks | partner
blocks] order, so the whole device program is static SPMD: slot j's
queries sit at column j*256, its key blocks at [0..j]*256 (own) and
2048 + [0..j]*256 (partner).  Per-core asymmetry lives in:
  - ecol: per-slot exp-constant columns that zero the filler block
    (scale=0 -> exp produces exact 0) when it is acausal,
  - host-side output reassembly (slot -> global block).

The diagonal (own pair j) uses one static triangular mask multiply.

Projections: QK fused as one [C,128] f16 weight, H-major.  V is
projected directly key-major (X^T chunks as the stationary operand),
which halves its PE cost and needs no transposes.  exp is split across
three engines: ACT computes exact exp; Pool/DVE use a one-instruction
Schraudolph fast-exp (int16 <- s*A + B, bitcast to f16, ~1.7% rms,
zero mean bias).  The softmax denominator rides as a ones-column in V;
O^T ([65, 256] numerator+denominator per slot) is DMA'd out raw and the
transpose + division happen on host.  The S->exp->O chain is software-
pipelined (depth 4) so the PE never waits on exp.
"""

import os

import numpy as np

import concourse.bacc as bacc
import concourse.mybir as mybir
import concourse.tile as tile
from concourse.bass_utils import run_bass_kernel_spmd

B, T, C, H = 4, 4096, 1024, 64
NCORES = 8
QW = 256  # query/key block width (ownership granularity)
KT = 128  # k tile (S^T partition dim)
CCH = C // 128  # contraction chunks
NSLOT = 8  # owned query blocks per core
NQ = NSLOT * QW  # queries per core (2048)
HALF = NSLOT * QW  # column offset of partner region (2048)
F32 = mybir.dt.float32
F16 = mybir.dt.float16
I16 = mybir.dt.int16
I32 = mybir.dt.int32

OWN = {
    0: [0, 3, 4, 7, 8, 11, 12, 15],
    1: [1, 2, 5, 6, 9, 10, 13, 14],
}

# fast-exp: i16 = int16(s_raw * FE_A + FE_B); bitcast f16 ~= exp(s_raw/8)
_LN2 = float(np.log(2.0))
FE_A = 1024.0 / _LN2 / 8.0
FE_B = 15.0 * 1024.0 - 59.5

DEPTH = 4  # software pipeline depth of the S->exp->O chain, in pairs

_PROGRAM = None


def _build_program():
    nc = bacc.Bacc(None, target_bir_lowering=False, debug=False)

    xt = nc.dram_tensor("xt", [C, T], F16, kind="ExternalInput")
    # weights pre-swizzled on host to [p, ci, m] so DMA runs are contiguous
    wqk = nc.dram_tensor("wqk", [128, CCH * 128], F16, kind="ExternalInput")
    wv = nc.dram_tensor("wv", [128, CCH * 64], F16, kind="ExternalInput")
    ecol = nc.dram_tensor("ecol", [128, 4 * NSLOT], F32, kind="ExternalInput")
    ot = nc.dram_tensor("ot", [H + 1, NQ], F32, kind="ExternalOutput")

    xt_r = xt.rearrange("(n p) t -> p n t", p=128)  # [128, 8, T]
    wqk_r = wqk.rearrange("p (n m) -> p n m", n=CCH)
    wv_r = wv.rearrange("p (n m) -> p n m", n=CCH)

    # exp engine schedule state: weighted rotation (ACT : DVE : Pool)
    sched = {"n": 0}

    def pick_exp_engine(no_pool=False):
        cyc = ["act", "act", "dve", "act", "act", "dve", "act", "act",
               "dve", "act", "act", "dve", "act", "act", "dve", "act",
               "act", "act"]
        e = cyc[sched["n"] % len(cyc)]
        sched["n"] += 1
        if no_pool and e == "pool":
            e = "act"
        return e

    with tile.TileContext(nc) as tc:
        with (
            tc.tile_pool(name="const", bufs=1) as const_pool,
            tc.tile_pool(name="big", bufs=1) as big_pool,
            tc.tile_pool(name="xin", bufs=6) as xin_pool,
            tc.tile_pool(name="p", bufs=DEPTH + 2) as p_pool,
            tc.tile_pool(name="outp", bufs=2) as out_pool,
            tc.tile_pool(name="ps_st", bufs=DEPTH, space="PSUM") as ps_st,
            tc.tile_pool(name="ps_proj", bufs=2, space="PSUM") as ps_proj,
            tc.tile_pool(name="ps_v", bufs=1, space="PSUM") as ps_v,
            tc.tile_pool(name="ps_o", bufs=1, space="PSUM") as ps_o,
        ):
            # ---- weights first (they gate the first matmul) ----
            wqk_s = const_pool.tile([128, CCH, 128], F16)
            nc.sync.dma_start(wqk_s[:], wqk_r)
            wv_s = const_pool.tile([128, CCH, 64], F16)
            nc.sync.dma_start(wv_s[:], wv_r)

            xin_tiles = {}

            def load_block(sb, split=False):
                """Start the X^T DMA for permuted columns
                [sb*256, (sb+1)*256)."""
                sl = slice(sb * QW, (sb + 1) * QW)
                xt_t = xin_pool.tile([128, CCH, QW], F16, tag="xin")
                if split:
                    h = CCH // 2
                    nc.sync.dma_start(xt_t[:, 0:h], xt_r[:, 0:h, sl])
                    nc.sync.dma_start(xt_t[:, h:CCH], xt_r[:, h:CCH, sl])
                else:
                    nc.sync.dma_start(xt_t[:], xt_r[:, :, sl])
                xin_tiles[sb] = xt_t

            load_block(0, split=True)
            load_block(NSLOT)

            # ---- remaining constants ----
            ecol_s = const_pool.tile([128, 4 * NSLOT], F32)
            nc.sync.dma_start(ecol_s[:], ecol[:])

            # static triangular mask for the diagonal pair:
            # mask01[kp, st, qf] = (qf - 128*st >= kp)
            iota_i = const_pool.tile([128, 2, QW], I32)
            nc.gpsimd.iota(
                iota_i[:], pattern=[[-KT, 2], [1, QW]], base=0,
                channel_multiplier=0,
            )
            iota_ff = const_pool.tile([128, 2, QW], F32)
            nc.vector.tensor_copy(iota_ff[:], iota_i[:])
            kp_i = const_pool.tile([128, 1], I32)
            nc.gpsimd.iota(kp_i[:], pattern=[[0, 1]], base=0,
                           channel_multiplier=1)
            kp_f = const_pool.tile([128, 1], F32)
            nc.vector.tensor_copy(kp_f[:], kp_i[:])
            mask01 = const_pool.tile([128, 2, QW], F16)
            nc.vector.tensor_scalar(
                mask01[:], iota_ff[:], kp_f[:, 0:1], None,
                mybir.AluOpType.is_ge,
            )

            # ---- persistent activations (own|partner permuted order) ----
            q_all = big_pool.tile([64, NQ], F16)  # own queries only
            kT_s = big_pool.tile([64, T], F16)
            v_s = big_pool.tile([128, T // KT, H + 1], F16)
            nc.vector.memset(v_s[:, :, H : H + 1], 1.0)

            def project_block(sb):
                """Project QK (H-major) and V (key-major) for permuted
                columns [sb*256, (sb+1)*256)."""
                sl = slice(sb * QW, (sb + 1) * QW)
                xt_t = xin_tiles.pop(sb)

                qk_ps = ps_proj.tile([128, QW], F32, tag="qk")
                for ci in range(CCH):
                    nc.tensor.matmul(
                        qk_ps[:], wqk_s[:, ci], xt_t[:, ci],
                        start=(ci == 0), stop=(ci == CCH - 1),
                    )
                if sb < NSLOT:  # own block: keep q
                    nc.vector.tensor_copy(q_all[:, sl], qk_ps[0:64, :])
                nc.vector.tensor_copy(kT_s[:, sl], qk_ps[64:128, :])

                v_ps = ps_v.tile([128, 2, 64], F32, tag="v")
                for g in range(2):
                    for ci in range(CCH):
                        nc.tensor.matmul(
                            v_ps[:, g],
                            xt_t[:, ci, g * 128 : (g + 1) * 128],
                            wv_s[:, ci],
                            start=(ci == 0), stop=(ci == CCH - 1),
                        )
                nc.vector.tensor_copy(
                    v_s[:, sb * 2 : sb * 2 + 2, 0:H], v_ps[:]
                )

            def attend_slot(j):
                """Slot j: queries q_all[:, j*256); key pairs: partner
                0..j (filler last), then own 0..j (diagonal last)."""
                q_ap = q_all[0:64, j * QW : (j + 1) * QW]
                o_ps = ps_o.tile([H + 1, QW], F32, tag="o")
                # pair list: (kT col base, kind).  In the final slot the
                # diagonal goes first so the drain tail isn't lengthened
                # by its exp->mask->O chain.
                pairs = [(HALF + i * QW, "filler" if i == j else "plain")
                         for i in range(j + 1)]
                own = [(i * QW, "diag" if i == j else "plain")
                       for i in range(j + 1)]
                if j == NSLOT - 1:
                    own = own[-1:] + own[:-1]
                pairs += own
                npairs = len(pairs)
                st_q = [None] * npairs
                p_q = [None] * npairs

                def emit_s(i):
                    base, _ = pairs[i]
                    st = ps_st.tile([128, 2, QW], F32, tag="st")
                    for s in range(2):
                        nc.tensor.matmul(
                            st[:, s],
                            kT_s[0:64, base + s * KT : base + (s + 1) * KT],
                            q_ap,
                            start=True, stop=True,
                        )
                    st_q[i] = st

                def emit_exp(i):
                    _, kind = pairs[i]
                    st = st_q[i]
                    p_sb = p_pool.tile([128, 2, QW], F16, tag="p")
                    eng = pick_exp_engine(no_pool=(kind == "diag"))
                    if kind == "filler":
                        if eng == "act":
                            nc.scalar.activation(
                                p_sb[:], st[:],
                                mybir.ActivationFunctionType.Exp,
                                scale=ecol_s[:, j : j + 1],
                                bias=ecol_s[:, NSLOT + j : NSLOT + j + 1],
                            )
                        else:
                            e = nc.vector
                            e.tensor_scalar(
                                p_sb.bitcast(I16)[:], st[:],
                                ecol_s[:, 2 * NSLOT + j : 2 * NSLOT + j + 1],
                                ecol_s[:, 3 * NSLOT + j : 3 * NSLOT + j + 1],
                                mybir.AluOpType.mult, mybir.AluOpType.add,
                            )
                    else:
                        if eng == "act":
                            nc.scalar.activation(
                                p_sb[:], st[:],
                                mybir.ActivationFunctionType.Exp,
                                scale=float(H) ** -0.5,
                            )
                        else:
                            e = nc.vector
                            e.tensor_scalar(
                                p_sb.bitcast(I16)[:], st[:],
                                FE_A, FE_B,
                                mybir.AluOpType.mult, mybir.AluOpType.add,
                            )
                    if kind == "diag":
                        nc.vector.tensor_tensor(
                            p_sb[:], p_sb[:], mask01[:],
                            mybir.AluOpType.mult,
                        )
                    p_q[i] = p_sb

                def emit_o(i):
                    base, _ = pairs[i]
                    p_sb = p_q[i]
                    for s in range(2):
                        vt = base // KT + s
                        nc.tensor.matmul(
                            o_ps[:], v_s[:, vt, :], p_sb[:, s],
                            start=(i == 0 and s == 0),
                            stop=(i == npairs - 1 and s == 1),
                        )

                # software pipeline: O(i) trails S/exp by DEPTH-1 pairs
                for i in range(npairs):
                    emit_s(i)
                    emit_exp(i)
                    if i >= DEPTH - 1:
                        emit_o(i - DEPTH + 1)
                for i in range(max(0, npairs - DEPTH + 1), npairs):
                    emit_o(i)

                ot_sb = out_pool.tile([H + 1, QW], F32, tag="ot")
                nc.vector.tensor_copy(ot_sb[:], o_ps[:])
                nc.sync.dma_start(ot[:, j * QW : (j + 1) * QW], ot_sb[:])

            # pipeline: DMA two iterations ahead, projection one ahead, so
            # projection copies enter the Pool/DVE queues before each
            # slot's exp backlog.
            load_block(1)
            load_block(NSLOT + 1)
            project_block(0)
            project_block(NSLOT)
            for j in range(NSLOT):
                if j + 2 < NSLOT:
                    load_block(j + 2)
                    load_block(NSLOT + j + 2)
                if j + 1 < NSLOT:
                    project_block(j + 1)
                    project_block(NSLOT + j + 1)
                attend_slot(j)

    nc.compile()
    return nc


def _ecol_table(half: int) -> np.ndarray:
    """Per-slot exp-constant columns.  Slot j's filler (partner pair j)
    is visible iff partner global block j precedes own global block j;
    otherwise its exp constants are zeroed so it contributes exact 0.
    Layout: [act_scale(8) | act_bias(8) | fe_a(8) | fe_b(8)]."""
    t = np.zeros((128, 4 * NSLOT), dtype=np.float32)
    own, part = OWN[half], OWN[1 - half]
    for j in range(NSLOT):
        vis = part[j] < own[j]
        t[:, j] = (float(H) ** -0.5) if vis else 0.0
        t[:, NSLOT + j] = 0.0 if vis else -100.0
        t[:, 2 * NSLOT + j] = FE_A if vis else 0.0
        t[:, 3 * NSLOT + j] = FE_B if vis else 0.0
    return t


def kernel(X, Wq, Wk, Wv):
    global _PROGRAM
    X = np.asarray(X, dtype=np.float32)
    Wq = np.asarray(Wq, dtype=np.float32)
    Wk = np.asarray(Wk, dtype=np.float32)
    Wv = np.asarray(Wv, dtype=np.float32)

    if _PROGRAM is None:
        _PROGRAM = _build_program()
    nc = _PROGRAM

    wqk = np.concatenate([Wq, Wk], axis=1)
    # device layout [p, ci, m]: w_sw[p, ci*M + m] = W[ci*128 + p, m]
    wqk_sw = np.ascontiguousarray(
        wqk.reshape(CCH, 128, 128).transpose(1, 0, 2).reshape(128, -1)
    ).astype(np.float16)
    wv_sw = np.ascontiguousarray(
        Wv.reshape(CCH, 128, 64).transpose(1, 0, 2).reshape(128, -1)
    ).astype(np.float16)

    in_maps = []
    for core in range(NCORES):
        b, half = core // 2, core % 2
        order = OWN[half] + OWN[1 - half]
        cols = np.concatenate(
            [np.arange(g * QW, (g + 1) * QW) for g in order]
        )
        in_maps.append(
            {
                "xt": np.ascontiguousarray(X[b].T[:, cols]).astype(np.float16),
                "wqk": wqk_sw,
                "wv": wv_sw,
                "ecol": _ecol_table(half),
            }
        )

    trace = bool(os.environ.get("KERNEL_TRACE"))
    if trace:
        try:
            from antenv.axon_hooks import get_axon_ntff_profile_hook  # noqa: F401
        except ImportError:
            print(
                "KERNEL_TRACE requested but axon NTFF hook unavailable; "
                "running untraced"
            )
            trace = False
    kwargs = {}
    if trace:
        kwargs = dict(
            trace=True,
            trace_cores=[
                int(c)
                for c in os.environ.get("KERNEL_TRACE_CORES", "0").split(",")
            ],
        )
    res = run_bass_kernel_spmd(nc, in_maps, core_ids=list(range(NCORES)), **kwargs)
    if trace:
        print(f"HW exec time: {res.exec_time_ns} ns")
        print(f"mean exec time: {res.mean_exec_time_ns} ns")
        kernel.last_results = res

    out = np.empty((B, T, H), dtype=np.float32)
    for core in range(NCORES):
        b, half = core // 2, core % 2
        oc = res.results[core]["ot"]  # [65, NQ]
        for j, g in enumerate(OWN[half]):
            blk = oc[:, j * QW : (j + 1) * QW]
            out[b, g * QW : (g + 1) * QW] = (blk[0:H] / blk[H : H + 1]).T
    return out


# revision 16
# speedup vs baseline: 1.5386x; 1.0720x over previous
"""Causal single-head attention (B=4, T=4096, C=1024, H=64) on 8 TRN2 cores.

Sharding: 2 cores per batch element, causal-balanced interleaved query
blocks of 256: half 0 owns global 256-blocks {0,3,4,7,8,11,12,15}, half 1
owns {1,2,5,6,9,10,13,14}.  With this ownership, a core's slot j (its
j-th owned block, ascending) attends exactly j+1 own blocks and
structurally j+1 partner blocks (one of which may be a fully-masked
filler) — the union is always the causal prefix, and the structural
shape is identical on every core (72 of the ideal 68 [256k x 256q]
tiles vs 80 for a 512-wide split).

The host PERMUTES each core's X^T columns into [own blocks | partner
blocks] order, so the whole device program is static SPMD: slot j's
queries sit at column j*256, its key blocks at [0..j]*256 (own) and
2048 + [0..j]*256 (partner).  Per-core asymmetry lives in:
  - ecol: per-slot exp-constant columns that zero the filler block
    (scale=0 -> exp produces exact 0) when it is acausal,
  - host-side output reassembly (slot -> global block).

The diagonal (own pair j) uses one static triangular mask multiply.

Projections: QK fused as one [C,128] f16 weight, H-major.  V is
projected directly key-major (X^T chunks as the stationary operand),
which halves its PE cost and needs no transposes.  exp is split across
three engines: ACT computes exact exp; Pool/DVE use a one-instruction
Schraudolph fast-exp (int16 <- s*A + B, bitcast to f16, ~1.7% rms,
zero mean bias).  The softmax denominator rides as a ones-column in V;
O^T ([65, 256] numerator+denominator per slot) is DMA'd out raw and the
transpose + division happen on host.  The S->exp->O chain is software-
pipelined (depth 4) so the PE never waits on exp.
"""

import os

import numpy as np

import concourse.bacc as bacc
import concourse.mybir as mybir
import concourse.tile as tile
from concourse.bass_utils import run_bass_kernel_spmd

B, T, C, H = 4, 4096, 1024, 64
NCORES = 8
QW = 256  # query/key block width (ownership granularity)
KT = 128  # k tile (S^T partition dim)
CCH = C // 128  # contraction chunks
NSLOT = 8  # owned query blocks per core
NQ = NSLOT * QW  # queries per core (2048)
HALF = NSLOT * QW  # column offset of partner region (2048)
F32 = mybir.dt.float32
F16 = mybir.dt.float16
I16 = mybir.dt.int16
I32 = mybir.dt.int32

OWN = {
    0: [0, 3, 4, 7, 8, 11, 12, 15],
    1: [1, 2, 5, 6, 9, 10, 13, 14],
}

WS = 32.0  # weight pre-scale so fp8 e4m3 covers W ~ N(0, C^-1)
SSC = float(H) ** -0.5 / (WS * WS)  # score scale: q,k both carry WS
# fast-exp: i16 = int16(s_raw * FE_A + FE_B); bitcast f16 ~= exp(s_raw*SSC)
_LN2 = float(np.log(2.0))
FE_A = 1024.0 / _LN2 * SSC
FE_B = 15.0 * 1024.0 - 59.5

DEPTH = 4  # software pipeline depth of the S->exp->O chain, in pairs

_PROGRAM = None


def _build_program():
    nc = bacc.Bacc(None, target_bir_lowering=False, debug=False)

    F8 = mybir.dt.float8e4
    xth = nc.dram_tensor("xth", [C, T], F8, kind="ExternalInput")
    xtl = nc.dram_tensor("xtl", [C, T], F8, kind="ExternalInput")
    # weights pre-swizzled on host to [p, ci, m] so DMA runs are contiguous
    wqkh = nc.dram_tensor("wqkh", [128, CCH * 128], F8, kind="ExternalInput")
    wqkl = nc.dram_tensor("wqkl", [128, CCH * 128], F8, kind="ExternalInput")
    wvh = nc.dram_tensor("wvh", [128, CCH * 64], F8, kind="ExternalInput")
    wvl = nc.dram_tensor("wvl", [128, CCH * 64], F8, kind="ExternalInput")
    ecol = nc.dram_tensor("ecol", [128, 4 * NSLOT], F32, kind="ExternalInput")
    ot = nc.dram_tensor("ot", [H + 1, NQ], F32, kind="ExternalOutput")

    xth_r = xth.rearrange("(n p) t -> p n t", p=128)  # [128, 8, T]
    xtl_r = xtl.rearrange("(n p) t -> p n t", p=128)
    wqkh_r = wqkh.rearrange("p (n m) -> p n m", n=CCH)
    wqkl_r = wqkl.rearrange("p (n m) -> p n m", n=CCH)
    wvh_r = wvh.rearrange("p (n m) -> p n m", n=CCH)
    wvl_r = wvl.rearrange("p (n m) -> p n m", n=CCH)

    # exp engine schedule state: weighted rotation (ACT : DVE : Pool)
    sched = {"n": 0}

    def pick_exp_engine(no_pool=False):
        cyc = ["act", "act", "dve", "act", "act", "dve", "act", "act",
               "dve", "act", "act", "dve", "act", "act", "dve", "act",
               "act", "act"]
        e = cyc[sched["n"] % len(cyc)]
        sched["n"] += 1
        if no_pool and e == "pool":
            e = "act"
        return e

    with tile.TileContext(nc) as tc:
        with (
            tc.tile_pool(name="const", bufs=1) as const_pool,
            tc.tile_pool(name="big", bufs=1) as big_pool,
            tc.tile_pool(name="xin", bufs=3) as xin_pool,
            tc.tile_pool(name="p", bufs=DEPTH + 2) as p_pool,
            tc.tile_pool(name="outp", bufs=2) as out_pool,
            tc.tile_pool(name="ps_st", bufs=DEPTH, space="PSUM") as ps_st,
            tc.tile_pool(name="ps_proj", bufs=2, space="PSUM") as ps_proj,
            tc.tile_pool(name="ps_v", bufs=1, space="PSUM") as ps_v,
            tc.tile_pool(name="ps_o", bufs=1, space="PSUM") as ps_o,
        ):
            # ---- weights first (they gate the first matmul) ----
            F8 = mybir.dt.float8e4
            wqkh_s = const_pool.tile([128, CCH, 128], F8)
            nc.sync.dma_start(wqkh_s[:], wqkh_r)

            xin_tiles = {}

            def load_pair(j, split=False):
                """Start the X^T hi/lo DMAs for permuted positions 2j
                (own block j) and 2j+1 (partner block j) as one 512-wide
                transfer each (512B dram runs keep full DMA rate)."""
                sl = slice(2 * j * QW, (2 * j + 2) * QW)
                xh_t = xin_pool.tile([128, CCH, 2 * QW], F8, tag="xinh")
                xl_t = xin_pool.tile([128, CCH, 2 * QW], F8, tag="xinl")
                if split:
                    h = CCH // 2
                    nc.sync.dma_start(xh_t[:, 0:h], xth_r[:, 0:h, sl])
                    nc.sync.dma_start(xh_t[:, h:CCH], xth_r[:, h:CCH, sl])
                else:
                    nc.sync.dma_start(xh_t[:], xth_r[:, :, sl])
                nc.sync.dma_start(xl_t[:], xtl_r[:, :, sl])
                xin_tiles[j] = (xh_t, xl_t)

            hcch = CCH // 2
            xh0 = xin_pool.tile([128, CCH, 2 * QW], F8, tag="xinh")
            nc.sync.dma_start(xh0[:, 0:hcch], xth_r[:, 0:hcch, 0 : 2 * QW])
            nc.sync.dma_start(xh0[:, hcch:CCH], xth_r[:, hcch:CCH, 0 : 2 * QW])
            wqkl_s = const_pool.tile([128, CCH, 128], F8)
            nc.sync.dma_start(wqkl_s[:], wqkl_r)
            xl0 = xin_pool.tile([128, CCH, 2 * QW], F8, tag="xinl")
            nc.sync.dma_start(xl0[:], xtl_r[:, :, 0 : 2 * QW])
            xin_tiles[0] = (xh0, xl0)
            wvh_s = const_pool.tile([128, CCH, 64], F8)
            nc.sync.dma_start(wvh_s[:], wvh_r)
            wvl_s = const_pool.tile([128, CCH, 64], F8)
            nc.sync.dma_start(wvl_s[:], wvl_r)

            # ---- remaining constants ----
            ecol_s = const_pool.tile([128, 4 * NSLOT], F32)
            nc.sync.dma_start(ecol_s[:], ecol[:])

            # static triangular mask for the diagonal pair:
            # mask01[kp, st, qf] = (qf - 128*st >= kp)
            iota_i = const_pool.tile([128, 2, QW], I32)
            nc.gpsimd.iota(
                iota_i[:], pattern=[[-KT, 2], [1, QW]], base=0,
                channel_multiplier=0,
            )
            iota_ff = const_pool.tile([128, 2, QW], F32)
            nc.vector.tensor_copy(iota_ff[:], iota_i[:])
            kp_i = const_pool.tile([128, 1], I32)
            nc.gpsimd.iota(kp_i[:], pattern=[[0, 1]], base=0,
                           channel_multiplier=1)
            kp_f = const_pool.tile([128, 1], F32)
            nc.vector.tensor_copy(kp_f[:], kp_i[:])
            mask01 = const_pool.tile([128, 2, QW], F16)
            nc.vector.tensor_scalar(
                mask01[:], iota_ff[:], kp_f[:, 0:1], None,
                mybir.AluOpType.is_ge,
            )

            # ---- persistent activations (own|partner permuted order) ----
            q_all = big_pool.tile([64, NQ], F16)  # own queries only
            kT_s = big_pool.tile([64, T], F16)
            v_s = big_pool.tile([128, T // KT, H + 1], F16)
            nc.vector.memset(v_s[:, :, H : H + 1], 1.0)

            def project_block(pos, last_of_pair):
                """Project QK (H-major) and V (key-major) for permuted
                position pos (even = own block pos//2, odd = partner)."""
                sl = slice(pos * QW, (pos + 1) * QW)
                j, hb = pos // 2, (pos % 2) * QW
                xh_p, xl_p = xin_tiles[j]
                xh_t = xh_p[:, :, hb : hb + QW]
                xl_t = xl_p[:, :, hb : hb + QW]
                if last_of_pair:
                    xin_tiles.pop(j)
                DR = mybir.MatmulPerfMode.DoubleRow
                NP = CCH // 2  # DoubleRow ci-pairs
                passes = [(wqkh_s, xh_t), (wqkl_s, xh_t), (wqkh_s, xl_t)]
                vpasses = [(wvh_s, xh_t), (wvl_s, xh_t), (wvh_s, xl_t)]

                qk_ps = ps_proj.tile([128, QW], F32, tag="qk")
                n = 0
                for w_s, x_t in passes:
                    for cp in range(NP):
                        nc.tensor.matmul(
                            qk_ps[:],
                            w_s[:, 2 * cp : 2 * cp + 2],
                            x_t[:, 2 * cp : 2 * cp + 2],
                            start=(n == 0), stop=(n == 3 * NP - 1),
                            perf_mode=DR,
                        )
                        n += 1
                if pos % 2 == 0:  # own block: keep q
                    qsl = slice((pos // 2) * QW, (pos // 2 + 1) * QW)
                    nc.vector.tensor_copy(q_all[:, qsl], qk_ps[0:64, :])
                nc.vector.tensor_copy(kT_s[:, sl], qk_ps[64:128, :])

                v_ps = ps_v.tile([128, 2, 64], F32, tag="v")
                for g in range(2):
                    n = 0
                    for w_s, x_t in vpasses:
                        for cp in range(NP):
                            nc.tensor.matmul(
                                v_ps[:, g],
                                x_t[:, 2 * cp : 2 * cp + 2,
                                    g * 128 : (g + 1) * 128],
                                w_s[:, 2 * cp : 2 * cp + 2],
                                start=(n == 0), stop=(n == 3 * NP - 1),
                                perf_mode=DR,
                            )
                            n += 1
                nc.vector.tensor_copy(
                    v_s[:, pos * 2 : pos * 2 + 2, 0:H], v_ps[:]
                )

            def attend_slot(j):
                """Slot j: queries q_all[:, j*256); key pairs: partner
                0..j (filler last), then own 0..j (diagonal last)."""
                q_ap = q_all[0:64, j * QW : (j + 1) * QW]
                o_ps = ps_o.tile([H + 1, QW], F32, tag="o")
                # pair list: (kT col base, kind).  In the final slot the
                # diagonal goes first so the drain tail isn't lengthened
                # by its exp->mask->O chain.
                pairs = [((2 * i + 1) * QW, "filler" if i == j else "plain")
                         for i in range(j + 1)]
                own = [(2 * i * QW, "diag" if i == j else "plain")
                       for i in range(j + 1)]
                if j == NSLOT - 1:
                    own = own[-1:] + own[:-1]
                pairs += own
                npairs = len(pairs)
                st_q = [None] * npairs
                p_q = [None] * npairs

                def emit_s(i):
                    base, _ = pairs[i]
                    st = ps_st.tile([128, 2, QW], F32, tag="st")
                    for s in range(2):
                        nc.tensor.matmul(
                            st[:, s],
                            kT_s[0:64, base + s * KT : base + (s + 1) * KT],
                            q_ap,
                            start=True, stop=True,
                        )
                    st_q[i] = st

                def emit_exp(i):
                    _, kind = pairs[i]
                    st = st_q[i]
                    p_sb = p_pool.tile([128, 2, QW], F16, tag="p")
                    eng = pick_exp_engine(no_pool=(kind == "diag"))
                    if kind == "filler":
                        if eng == "act":
                            nc.scalar.activation(
                                p_sb[:], st[:],
                                mybir.ActivationFunctionType.Exp,
                                scale=ecol_s[:, j : j + 1],
                                bias=ecol_s[:, NSLOT + j : NSLOT + j + 1],
                            )
                        else:
                            e = nc.vector
                            e.tensor_scalar(
                                p_sb.bitcast(I16)[:], st[:],
                                ecol_s[:, 2 * NSLOT + j : 2 * NSLOT + j + 1],
                                ecol_s[:, 3 * NSLOT + j : 3 * NSLOT + j + 1],
                                mybir.AluOpType.mult, mybir.AluOpType.add,
                            )
                    else:
                        if eng == "act":
                            nc.scalar.activation(
                                p_sb[:], st[:],
                                mybir.ActivationFunctionType.Exp,
                                scale=SSC,
                            )
                        else:
                            e = nc.vector
                            e.tensor_scalar(
                                p_sb.bitcast(I16)[:], st[:],
                                FE_A, FE_B,
                                mybir.AluOpType.mult, mybir.AluOpType.add,
                            )
                    if kind == "diag":
                        nc.vector.tensor_tensor(
                            p_sb[:], p_sb[:], mask01[:],
                            mybir.AluOpType.mult,
                        )
                    p_q[i] = p_sb

                def emit_o(i):
                    base, _ = pairs[i]
                    p_sb = p_q[i]
                    for s in range(2):
                        vt = base // KT + s
                        nc.tensor.matmul(
                            o_ps[:], v_s[:, vt, :], p_sb[:, s],
                            start=(i == 0 and s == 0),
                            stop=(i == npairs - 1 and s == 1),
                        )

                # software pipeline: O(i) trails S/exp by DEPTH-1 pairs
                for i in range(npairs):
                    emit_s(i)
                    emit_exp(i)
                    if i >= DEPTH - 1:
                        emit_o(i - DEPTH + 1)
                for i in range(max(0, npairs - DEPTH + 1), npairs):
                    emit_o(i)

                ot_sb = out_pool.tile([H + 1, QW], F32, tag="ot")
                nc.vector.tensor_copy(ot_sb[:], o_ps[:])
                nc.sync.dma_start(ot[:, j * QW : (j + 1) * QW], ot_sb[:])

            # pipeline: DMA two iterations ahead, projection one ahead, so
            # projection copies enter the Pool/DVE queues before each
            # slot's exp backlog.
            load_pair(1)
            project_block(0, False)
            project_block(1, True)
            for j in range(NSLOT):
                if j + 2 < NSLOT:
                    load_pair(j + 2)
                if j + 1 < NSLOT:
                    project_block(2 * (j + 1), False)
                    project_block(2 * (j + 1) + 1, True)
                attend_slot(j)

    nc.compile()
    return nc


def _ecol_table(half: int) -> np.ndarray:
    """Per-slot exp-constant columns.  Slot j's filler (partner pair j)
    is visible iff partner global block j precedes own global block j;
    otherwise its exp constants are zeroed so it contributes exact 0.
    Layout: [act_scale(8) | act_bias(8) | fe_a(8) | fe_b(8)]."""
    t = np.zeros((128, 4 * NSLOT), dtype=np.float32)
    own, part = OWN[half], OWN[1 - half]
    for j in range(NSLOT):
        vis = part[j] < own[j]
        t[:, j] = SSC if vis else 0.0
        t[:, NSLOT + j] = 0.0 if vis else -100.0
        t[:, 2 * NSLOT + j] = FE_A if vis else 0.0
        t[:, 3 * NSLOT + j] = FE_B if vis else 0.0
    return t


def kernel(X, Wq, Wk, Wv):
    global _PROGRAM
    X = np.asarray(X, dtype=np.float32)
    Wq = np.asarray(Wq, dtype=np.float32)
    Wk = np.asarray(Wk, dtype=np.float32)
    Wv = np.asarray(Wv, dtype=np.float32)

    if _PROGRAM is None:
        _PROGRAM = _build_program()
    nc = _PROGRAM

    import ml_dtypes
    E4 = ml_dtypes.float8_e4m3fn

    def q8(x):
        return np.asarray(x).astype(E4)

    def swiz(w, m):
        return np.ascontiguousarray(
            w.reshape(CCH, 128, m).transpose(1, 0, 2).reshape(128, -1)
        )

    wqk = np.concatenate([Wq, Wk], axis=1) * WS
    wqk_h = q8(wqk)
    wqk_l = q8(wqk - wqk_h.astype(np.float32))
    wv = Wv * WS
    wv_h = q8(wv)
    wv_l = q8(wv - wv_h.astype(np.float32))
    wqkh_sw, wqkl_sw = swiz(wqk_h, 128), swiz(wqk_l, 128)
    wvh_sw, wvl_sw = swiz(wv_h, 64), swiz(wv_l, 64)

    in_maps = []
    for core in range(NCORES):
        b, half = core // 2, core % 2
        order = [g for p in zip(OWN[half], OWN[1 - half]) for g in p]
        cols = np.concatenate(
            [np.arange(g * QW, (g + 1) * QW) for g in order]
        )
        xp = X[b].T[:, cols]
        xp_h = q8(xp)
        xp_l = q8(xp - xp_h.astype(np.float32))
        in_maps.append(
            {
                "xth": np.ascontiguousarray(xp_h),
                "xtl": np.ascontiguousarray(xp_l),
                "wqkh": wqkh_sw,
                "wqkl": wqkl_sw,
                "wvh": wvh_sw,
                "wvl": wvl_sw,
                "ecol": _ecol_table(half),
            }
        )

    trace = bool(os.environ.get("KERNEL_TRACE"))
    if trace:
        try:
            from antenv.axon_hooks import get_axon_ntff_profile_hook  # noqa: F401
        except ImportError:
            print(
                "KERNEL_TRACE requested but axon NTFF hook unavailable; "
                "running untraced"
            )
            trace = False
    kwargs = {}
    if trace:
        kwargs = dict(
            trace=True,
            trace_cores=[
                int(c)
                for c in os.environ.get("KERNEL_TRACE_CORES", "0").split(",")
            ],
        )
    res = run_bass_kernel_spmd(nc, in_maps, core_ids=list(range(NCORES)), **kwargs)
    if trace:
        print(f"HW exec time: {res.exec_time_ns} ns")
        print(f"mean exec time: {res.mean_exec_time_ns} ns")
        kernel.last_results = res

    out = np.empty((B, T, H), dtype=np.float32)
    for core in range(NCORES):
        b, half = core // 2, core % 2
        oc = res.results[core]["ot"]  # [65, NQ]
        for j, g in enumerate(OWN[half]):
            blk = oc[:, j * QW : (j + 1) * QW]
            out[b, g * QW : (g + 1) * QW] = (blk[0:H] / (blk[H : H + 1] * WS)).T
    return out


# revision 18
# speedup vs baseline: 1.5738x; 1.0229x over previous
"""Causal single-head attention (B=4, T=4096, C=1024, H=64) on 8 TRN2 cores.

Sharding: 2 cores per batch element, causal-balanced interleaved query
blocks of 256: half 0 owns global 256-blocks {0,3,4,7,8,11,12,15}, half 1
owns {1,2,5,6,9,10,13,14}.  With this ownership, a core's slot j (its
j-th owned block, ascending) attends exactly j+1 own blocks and
structurally j+1 partner blocks (one of which may be a fully-masked
filler) — the union is always the causal prefix, and the structural
shape is identical on every core (72 of the ideal 68 [256k x 256q]
tiles vs 80 for a 512-wide split).

The host PERMUTES each core's X^T columns into [own blocks | partner
blocks] order, so the whole device program is static SPMD: slot j's
queries sit at column j*256, its key blocks at [0..j]*256 (own) and
2048 + [0..j]*256 (partner).  Per-core asymmetry lives in:
  - ecol: per-slot exp-constant columns that zero the filler block
    (scale=0 -> exp produces exact 0) when it is acausal,
  - host-side output reassembly (slot -> global block).

The diagonal (own pair j) uses one static triangular mask multiply.

Projections run as 3-pass error-compensated fp8 DoubleRow matmuls
(Xh@Wh + Xh@Wl + Xl@Wh, hi/lo splits prepared on host, weights
pre-scaled x32 so W ~ N(0, C^-1) clears e4m3's subnormal range) at
0.5 cycles/row — 4x the f16 rate.  QK is fused as one [C,128] weight,
H-major; V is projected directly key-major (X^T chunks as the
stationary operand), which halves its PE cost and needs no transposes.
exp is split across ACT (exact) and DVE (one-instruction Schraudolph
fast-exp: int16 <- s*A + B, bitcast to f16, ~1.7% rms, zero mean
bias); Pool/GPSIMD cannot touch PSUM so it only builds constants.
The softmax denominator rides as a ones-column in V;
O^T ([65, 256] numerator+denominator per slot) is DMA'd out raw and the
transpose + division happen on host.  The S->exp->O chain is software-
pipelined (depth 4) so the PE never waits on exp.
"""

import os

import numpy as np

import concourse.bacc as bacc
import concourse.mybir as mybir
import concourse.tile as tile
from concourse.bass_utils import run_bass_kernel_spmd

B, T, C, H = 4, 4096, 1024, 64
NCORES = 8
QW = 256  # query/key block width (ownership granularity)
KT = 128  # k tile (S^T partition dim)
CCH = C // 128  # contraction chunks
NSLOT = 8  # owned query blocks per core
NQ = NSLOT * QW  # queries per core (2048)
HALF = NSLOT * QW  # column offset of partner region (2048)
F32 = mybir.dt.float32
F16 = mybir.dt.float16
I16 = mybir.dt.int16
I32 = mybir.dt.int32

OWN = {
    0: [0, 3, 4, 7, 8, 11, 12, 15],
    1: [1, 2, 5, 6, 9, 10, 13, 14],
}

WS = 32.0  # weight pre-scale so fp8 e4m3 covers W ~ N(0, C^-1)
SSC = float(H) ** -0.5 / (WS * WS)  # score scale: q,k both carry WS
# fast-exp: i16 = int16(s_raw * FE_A + FE_B); bitcast f16 ~= exp(s_raw*SSC)
_LN2 = float(np.log(2.0))
FE_A = 1024.0 / _LN2 * SSC
FE_B = 15.0 * 1024.0 - 59.5

DEPTH = 4  # software pipeline depth of the S->exp->O chain, in pairs

_PROGRAM = None


def _build_program():
    nc = bacc.Bacc(None, target_bir_lowering=False, debug=False)

    F8 = mybir.dt.float8e4
    xth = nc.dram_tensor("xth", [C, T], F8, kind="ExternalInput")
    xtl = nc.dram_tensor("xtl", [C, T], F8, kind="ExternalInput")
    # weights pre-swizzled on host to [p, ci, m] so DMA runs are contiguous
    wqkh = nc.dram_tensor("wqkh", [128, CCH * 128], F8, kind="ExternalInput")
    wqkl = nc.dram_tensor("wqkl", [128, CCH * 128], F8, kind="ExternalInput")
    wvh = nc.dram_tensor("wvh", [128, CCH * 64], F8, kind="ExternalInput")
    wvl = nc.dram_tensor("wvl", [128, CCH * 64], F8, kind="ExternalInput")
    ecol = nc.dram_tensor("ecol", [128, 4 * NSLOT], F32, kind="ExternalInput")
    ot = nc.dram_tensor("ot", [H + 1, NQ], F32, kind="ExternalOutput")

    xth_r = xth.rearrange("(n p) t -> p n t", p=128)  # [128, 8, T]
    xtl_r = xtl.rearrange("(n p) t -> p n t", p=128)
    wqkh_r = wqkh.rearrange("p (n m) -> p n m", n=CCH)
    wqkl_r = wqkl.rearrange("p (n m) -> p n m", n=CCH)
    wvh_r = wvh.rearrange("p (n m) -> p n m", n=CCH)
    wvl_r = wvl.rearrange("p (n m) -> p n m", n=CCH)

    # exp engine schedule state: weighted rotation (ACT : DVE : Pool)
    sched = {"n": 0}

    def pick_exp_engine(no_pool=False):
        cyc = ["act", "act", "dve", "act", "act", "dve", "act", "act",
               "dve", "act", "act", "dve", "act", "act", "dve", "act",
               "act", "act"]
        e = cyc[sched["n"] % len(cyc)]
        sched["n"] += 1
        if no_pool and e == "pool":
            e = "act"
        return e

    with tile.TileContext(nc) as tc:
        with (
            tc.tile_pool(name="const", bufs=1) as const_pool,
            tc.tile_pool(name="big", bufs=1) as big_pool,
            tc.tile_pool(name="xin", bufs=3) as xin_pool,
            tc.tile_pool(name="p", bufs=DEPTH + 2) as p_pool,
            tc.tile_pool(name="outp", bufs=2) as out_pool,
            tc.tile_pool(name="ps_st", bufs=DEPTH, space="PSUM") as ps_st,
            tc.tile_pool(name="ps_proj", bufs=2, space="PSUM") as ps_proj,
            tc.tile_pool(name="ps_v", bufs=1, space="PSUM") as ps_v,
            tc.tile_pool(name="ps_o", bufs=1, space="PSUM") as ps_o,
        ):
            # ---- weights first (they gate the first matmul) ----
            F8 = mybir.dt.float8e4
            wqkh_s = const_pool.tile([128, CCH, 128], F8)
            nc.sync.dma_start(wqkh_s[:], wqkh_r)

            xin_tiles = {}

            def load_pair(j, split=False):
                """Start the X^T hi/lo DMAs for permuted positions 2j
                (own block j) and 2j+1 (partner block j) as one 512-wide
                transfer each (512B dram runs keep full DMA rate)."""
                sl = slice(2 * j * QW, (2 * j + 2) * QW)
                xh_t = xin_pool.tile([128, CCH, 2 * QW], F8, tag="xinh")
                xl_t = xin_pool.tile([128, CCH, 2 * QW], F8, tag="xinl")
                if split:
                    h = CCH // 2
                    nc.sync.dma_start(xh_t[:, 0:h], xth_r[:, 0:h, sl])
                    nc.sync.dma_start(xh_t[:, h:CCH], xth_r[:, h:CCH, sl])
                else:
                    nc.sync.dma_start(xh_t[:], xth_r[:, :, sl])
                nc.sync.dma_start(xl_t[:], xtl_r[:, :, sl])
                xin_tiles[j] = (xh_t, xl_t)

            hcch = CCH // 2
            xh0 = xin_pool.tile([128, CCH, 2 * QW], F8, tag="xinh")
            nc.sync.dma_start(xh0[:, 0:hcch], xth_r[:, 0:hcch, 0 : 2 * QW])
            nc.sync.dma_start(xh0[:, hcch:CCH], xth_r[:, hcch:CCH, 0 : 2 * QW])
            wqkl_s = const_pool.tile([128, CCH, 128], F8)
            nc.sync.dma_start(wqkl_s[:], wqkl_r)
            xl0 = xin_pool.tile([128, CCH, 2 * QW], F8, tag="xinl")
            nc.sync.dma_start(xl0[:], xtl_r[:, :, 0 : 2 * QW])
            xin_tiles[0] = (xh0, xl0)
            wvh_s = const_pool.tile([128, CCH, 64], F8)
            nc.sync.dma_start(wvh_s[:], wvh_r)
            wvl_s = const_pool.tile([128, CCH, 64], F8)
            nc.sync.dma_start(wvl_s[:], wvl_r)

            # ---- remaining constants ----
            ecol_s = const_pool.tile([128, 4 * NSLOT], F32)
            nc.sync.dma_start(ecol_s[:], ecol[:])

            # static triangular mask for the diagonal pair:
            # mask01[kp, st, qf] = (qf - 128*st >= kp)
            iota_i = const_pool.tile([128, 2, QW], I32)
            nc.gpsimd.iota(
                iota_i[:], pattern=[[-KT, 2], [1, QW]], base=0,
                channel_multiplier=0,
            )
            iota_ff = const_pool.tile([128, 2, QW], F32)
            nc.vector.tensor_copy(iota_ff[:], iota_i[:])
            kp_i = const_pool.tile([128, 1], I32)
            nc.gpsimd.iota(kp_i[:], pattern=[[0, 1]], base=0,
                           channel_multiplier=1)
            kp_f = const_pool.tile([128, 1], F32)
            nc.vector.tensor_copy(kp_f[:], kp_i[:])
            mask01 = const_pool.tile([128, 2, QW], F16)
            nc.vector.tensor_scalar(
                mask01[:], iota_ff[:], kp_f[:, 0:1], None,
                mybir.AluOpType.is_ge,
            )

            # ---- persistent activations (own|partner permuted order) ----
            q_all = big_pool.tile([64, NQ], F16)  # own queries only
            kT_s = big_pool.tile([64, T], F16)
            v_s = big_pool.tile([128, T // KT, H + 1], F16)
            nc.vector.memset(v_s[:, :, H : H + 1], 1.0)

            proj_chunks = []

            def queue_project_block(pos, last_of_pair):
                """Queue chunked QK+V projection work for permuted
                position pos; chunks are emitted interleaved into the
                attend pair stream so proj matmuls fill the PE while
                exp results are pending."""
                sl = slice(pos * QW, (pos + 1) * QW)
                j, hb = pos // 2, (pos % 2) * QW
                xh_p, xl_p = xin_tiles[j]
                xh_t = xh_p[:, :, hb : hb + QW]
                xl_t = xl_p[:, :, hb : hb + QW]
                if last_of_pair:
                    xin_tiles.pop(j)
                DR = mybir.MatmulPerfMode.DoubleRow
                NP = CCH // 2  # DoubleRow ci-pairs
                passes = [(wqkh_s, xh_t), (wqkl_s, xh_t), (wqkh_s, xl_t)]
                vpasses = [(wvh_s, xh_t), (wvl_s, xh_t), (wvh_s, xl_t)]
                state = {}

                def qk_chunk(pi):
                    def emit():
                        if pi == 0:
                            state["qk"] = ps_proj.tile(
                                [128, QW], F32, tag="qk",
                                name=f"qk_{pos}",
                            )
                        qk_ps = state["qk"]
                        w_s, x_t = passes[pi]
                        for cp in range(NP):
                            nc.tensor.matmul(
                                qk_ps[:],
                                w_s[:, 2 * cp : 2 * cp + 2],
                                x_t[:, 2 * cp : 2 * cp + 2],
                                start=(pi == 0 and cp == 0),
                                stop=(pi == 2 and cp == NP - 1),
                                perf_mode=DR,
                            )
                        if pi == 2:
                            if pos % 2 == 0:  # own block: keep q
                                qsl = slice((pos // 2) * QW,
                                            (pos // 2 + 1) * QW)
                                nc.vector.tensor_copy(
                                    q_all[:, qsl], qk_ps[0:64, :]
                                )
                            nc.vector.tensor_copy(
                                kT_s[:, sl], qk_ps[64:128, :]
                            )
                    return emit

                def v_chunk(g):
                    def emit():
                        if g == 0:
                            state["v"] = ps_v.tile(
                                [128, 2, 64], F32, tag="v",
                                name=f"v_{pos}",
                            )
                        v_ps = state["v"]
                        n = 0
                        for w_s, x_t in vpasses:
                            for cp in range(NP):
                                nc.tensor.matmul(
                                    v_ps[:, g],
                                    x_t[:, 2 * cp : 2 * cp + 2,
                                        g * 128 : (g + 1) * 128],
                                    w_s[:, 2 * cp : 2 * cp + 2],
                                    start=(n == 0),
                                    stop=(n == 3 * NP - 1),
                                    perf_mode=DR,
                                )
                                n += 1
                        if g == 1:
                            nc.vector.tensor_copy(
                                v_s[:, pos * 2 : pos * 2 + 2, 0:H], v_ps[:]
                            )
                    return emit

                for pi in range(3):
                    proj_chunks.append(qk_chunk(pi))
                for g in range(2):
                    proj_chunks.append(v_chunk(g))

            def project_block(pos, last_of_pair):
                queue_project_block(pos, last_of_pair)
                while proj_chunks:
                    proj_chunks.pop(0)()

            def attend_slot(j):
                """Slot j: queries q_all[:, j*256); key pairs: partner
                0..j (filler last), then own 0..j (diagonal last)."""
                q_ap = q_all[0:64, j * QW : (j + 1) * QW]
                o_ps = ps_o.tile([H + 1, QW], F32, tag="o")
                # pair list: (kT col base, kind).  In the final slot the
                # diagonal goes first so the drain tail isn't lengthened
                # by its exp->mask->O chain.
                pairs = [((2 * i + 1) * QW, "filler" if i == j else "plain")
                         for i in range(j + 1)]
                own = [(2 * i * QW, "diag" if i == j else "plain")
                       for i in range(j + 1)]
                if j == NSLOT - 1:
                    own = own[-1:] + own[:-1]
                pairs += own
                npairs = len(pairs)
                st_q = [None] * npairs
                p_q = [None] * npairs

                def emit_s(i):
                    base, _ = pairs[i]
                    st = ps_st.tile([128, 2, QW], F32, tag="st")
                    for s in range(2):
                        nc.tensor.matmul(
                            st[:, s],
                            kT_s[0:64, base + s * KT : base + (s + 1) * KT],
                            q_ap,
                            start=True, stop=True,
                        )
                    st_q[i] = st

                def emit_exp(i):
                    _, kind = pairs[i]
                    st = st_q[i]
                    p_sb = p_pool.tile([128, 2, QW], F16, tag="p")
                    eng = pick_exp_engine(no_pool=(kind == "diag"))
                    if kind == "filler":
                        if eng == "act":
                            nc.scalar.activation(
                                p_sb[:], st[:],
                                mybir.ActivationFunctionType.Exp,
                                scale=ecol_s[:, j : j + 1],
                                bias=ecol_s[:, NSLOT + j : NSLOT + j + 1],
                            )
                        else:
                            e = nc.vector
                            e.tensor_scalar(
                                p_sb.bitcast(I16)[:], st[:],
                                ecol_s[:, 2 * NSLOT + j : 2 * NSLOT + j + 1],
                                ecol_s[:, 3 * NSLOT + j : 3 * NSLOT + j + 1],
                                mybir.AluOpType.mult, mybir.AluOpType.add,
                            )
                    else:
                        if eng == "act":
                            nc.scalar.activation(
                                p_sb[:], st[:],
                                mybir.ActivationFunctionType.Exp,
                                scale=SSC,
                            )
                        else:
                            e = nc.vector
                            e.tensor_scalar(
                                p_sb.bitcast(I16)[:], st[:],
                                FE_A, FE_B,
                                mybir.AluOpType.mult, mybir.AluOpType.add,
                            )
                    if kind == "diag":
                        nc.vector.tensor_tensor(
                            p_sb[:], p_sb[:], mask01[:],
                            mybir.AluOpType.mult,
                        )
                    p_q[i] = p_sb

                def emit_o(i):
                    base, _ = pairs[i]
                    p_sb = p_q[i]
                    for s in range(2):
                        vt = base // KT + s
                        nc.tensor.matmul(
                            o_ps[:], v_s[:, vt, :], p_sb[:, s],
                            start=(i == 0 and s == 0),
                            stop=(i == npairs - 1 and s == 1),
                        )

                # software pipeline: O(i) trails S/exp by DEPTH-1 pairs;
                # queued projection chunks fill the PE in exp windows
                for i in range(npairs):
                    emit_s(i)
                    emit_exp(i)
                    if proj_chunks:
                        proj_chunks.pop(0)()
                    if i >= DEPTH - 1:
                        emit_o(i - DEPTH + 1)
                while proj_chunks:
                    proj_chunks.pop(0)()
                for i in range(max(0, npairs - DEPTH + 1), npairs):
                    emit_o(i)

                ot_sb = out_pool.tile([H + 1, QW], F32, tag="ot")
                nc.vector.tensor_copy(ot_sb[:], o_ps[:])
                nc.sync.dma_start(ot[:, j * QW : (j + 1) * QW], ot_sb[:])

            # pipeline: DMA two iterations ahead, projection one ahead, so
            # projection copies enter the Pool/DVE queues before each
            # slot's exp backlog.
            load_pair(1)
            project_block(0, False)
            project_block(1, True)
            for j in range(NSLOT):
                if j + 2 < NSLOT:
                    load_pair(j + 2)
                if j + 1 < NSLOT:
                    queue_project_block(2 * (j + 1), False)
                    queue_project_block(2 * (j + 1) + 1, True)
                attend_slot(j)

    nc.compile()
    return nc


def _ecol_table(half: int) -> np.ndarray:
    """Per-slot exp-constant columns.  Slot j's filler (partner pair j)
    is visible iff partner global block j precedes own global block j;
    otherwise its exp constants are zeroed so it contributes exact 0.
    Layout: [act_scale(8) | act_bias(8) | fe_a(8) | fe_b(8)]."""
    t = np.zeros((128, 4 * NSLOT), dtype=np.float32)
    own, part = OWN[half], OWN[1 - half]
    for j in range(NSLOT):
        vis = part[j] < own[j]
        t[:, j] = SSC if vis else 0.0
        t[:, NSLOT + j] = 0.0 if vis else -100.0
        t[:, 2 * NSLOT + j] = FE_A if vis else 0.0
        t[:, 3 * NSLOT + j] = FE_B if vis else 0.0
    return t


def kernel(X, Wq, Wk, Wv):
    global _PROGRAM
    X = np.asarray(X, dtype=np.float32)
    Wq = np.asarray(Wq, dtype=np.float32)
    Wk = np.asarray(Wk, dtype=np.float32)
    Wv = np.asarray(Wv, dtype=np.float32)

    if _PROGRAM is None:
        _PROGRAM = _build_program()
    nc = _PROGRAM

    import ml_dtypes
    E4 = ml_dtypes.float8_e4m3fn

    def q8(x):
        return np.asarray(x).astype(E4)

    def swiz(w, m):
        return np.ascontiguousarray(
            w.reshape(CCH, 128, m).transpose(1, 0, 2).reshape(128, -1)
        )

    wqk = np.concatenate([Wq, Wk], axis=1) * WS
    wqk_h = q8(wqk)
    wqk_l = q8(wqk - wqk_h.astype(np.float32))
    wv = Wv * WS
    wv_h = q8(wv)
    wv_l = q8(wv - wv_h.astype(np.float32))
    wqkh_sw, wqkl_sw = swiz(wqk_h, 128), swiz(wqk_l, 128)
    wvh_sw, wvl_sw = swiz(wv_h, 64), swiz(wv_l, 64)

    in_maps = []
    for core in range(NCORES):
        b, half = core // 2, core % 2
        order = [g for p in zip(OWN[half], OWN[1 - half]) for g in p]
        cols = np.concatenate(
            [np.arange(g * QW, (g + 1) * QW) for g in order]
        )
        xp = X[b].T[:, cols]
        xp_h = q8(xp)
        xp_l = q8(xp - xp_h.astype(np.float32))
        in_maps.append(
            {
                "xth": np.ascontiguousarray(xp_h),
                "xtl": np.ascontiguousarray(xp_l),
                "wqkh": wqkh_sw,
                "wqkl": wqkl_sw,
                "wvh": wvh_sw,
                "wvl": wvl_sw,
                "ecol": _ecol_table(half),
            }
        )

    trace = bool(os.environ.get("KERNEL_TRACE"))
    if trace:
        try:
            from antenv.axon_hooks import get_axon_ntff_profile_hook  # noqa: F401
        except ImportError:
            print(
                "KERNEL_TRACE requested but axon NTFF hook unavailable; "
                "running untraced"
            )
            trace = False
    kwargs = {}
    if trace:
        kwargs = dict(
            trace=True,
            trace_cores=[
                int(c)
                for c in os.environ.get("KERNEL_TRACE_CORES", "0").split(",")
            ],
        )
    res = run_bass_kernel_spmd(nc, in_maps, core_ids=list(range(NCORES)), **kwargs)
    if trace:
        print(f"HW exec time: {res.exec_time_ns} ns")
        print(f"mean exec time: {res.mean_exec_time_ns} ns")
        kernel.last_results = res

    out = np.empty((B, T, H), dtype=np.float32)
    for core in range(NCORES):
        b, half = core // 2, core % 2
        oc = res.results[core]["ot"]  # [65, NQ]
        for j, g in enumerate(OWN[half]):
            blk = oc[:, j * QW : (j + 1) * QW]
            out[b, g * QW : (g + 1) * QW] = (blk[0:H] / (blk[H : H + 1] * WS)).T
    return out


# revision 19
# speedup vs baseline: 1.5855x; 1.0074x over previous
"""Causal single-head attention (B=4, T=4096, C=1024, H=64) on 8 TRN2 cores.

Sharding: 2 cores per batch element, causal-balanced interleaved query
blocks of 256: half 0 owns global 256-blocks {0,3,4,7,8,11,12,15}, half 1
owns {1,2,5,6,9,10,13,14}.  With this ownership, a core's slot j (its
j-th owned block, ascending) attends exactly j+1 own blocks and
structurally j+1 partner blocks (one of which may be a fully-masked
filler) — the union is always the causal prefix, and the structural
shape is identical on every core (72 of the ideal 68 [256k x 256q]
tiles vs 80 for a 512-wide split).

The host PERMUTES each core's X^T columns into [own blocks | partner
blocks] order, so the whole device program is static SPMD: slot j's
queries sit at column j*256, its key blocks at [0..j]*256 (own) and
2048 + [0..j]*256 (partner).  Per-core asymmetry lives in:
  - ecol: per-slot exp-constant columns that zero the filler block
    (scale=0 -> exp produces exact 0) when it is acausal,
  - host-side output reassembly (slot -> global block).

The diagonal (own pair j) uses one static triangular mask multiply.

Projections run as 3-pass error-compensated fp8 DoubleRow matmuls
(Xh@Wh + Xh@Wl + Xl@Wh, hi/lo splits prepared on host, weights
pre-scaled x32 so W ~ N(0, C^-1) clears e4m3's subnormal range) at
0.5 cycles/row — 4x the f16 rate.  QK is fused as one [C,128] weight,
H-major; V is projected directly key-major (X^T chunks as the
stationary operand), which halves its PE cost and needs no transposes.
exp is split across ACT (exact) and DVE (one-instruction Schraudolph
fast-exp: int16 <- s*A + B, bitcast to f16, ~1.7% rms, zero mean
bias); Pool/GPSIMD cannot touch PSUM so it only builds constants.
The softmax denominator rides as a ones-column in V;
O^T ([65, 256] numerator+denominator per slot) is DMA'd out raw and the
transpose + division happen on host.  The S->exp->O chain is software-
pipelined (depth 4) so the PE never waits on exp.
"""

import os

import numpy as np

import concourse.bacc as bacc
import concourse.mybir as mybir
import concourse.tile as tile
from concourse.bass_utils import run_bass_kernel_spmd

B, T, C, H = 4, 4096, 1024, 64
NCORES = 8
QW = 256  # query/key block width (ownership granularity)
KT = 128  # k tile (S^T partition dim)
CCH = C // 128  # contraction chunks
NSLOT = 8  # owned query blocks per core
NQ = NSLOT * QW  # queries per core (2048)
HALF = NSLOT * QW  # column offset of partner region (2048)
F32 = mybir.dt.float32
F16 = mybir.dt.float16
I16 = mybir.dt.int16
I32 = mybir.dt.int32

OWN = {
    0: [0, 3, 4, 7, 8, 11, 12, 15],
    1: [1, 2, 5, 6, 9, 10, 13, 14],
}

WS = 32.0  # weight pre-scale so fp8 e4m3 covers W ~ N(0, C^-1)
SSC = float(H) ** -0.5 / (WS * WS)  # score scale: q,k both carry WS
# fast-exp: i16 = int16(s_raw * FE_A + FE_B); bitcast f16 ~= exp(s_raw*SSC)
_LN2 = float(np.log(2.0))
FE_A = 1024.0 / _LN2 * SSC
FE_B = 15.0 * 1024.0 - 59.5

DEPTH = 4  # software pipeline depth of the S->exp->O chain, in pairs

_PROGRAM = None


def _build_program():
    nc = bacc.Bacc(None, target_bir_lowering=False, debug=False)

    F8 = mybir.dt.float8e4
    xth = nc.dram_tensor("xth", [C, T], F8, kind="ExternalInput")
    xtl = nc.dram_tensor("xtl", [C, T], F8, kind="ExternalInput")
    # weights pre-swizzled on host to [p, ci, m] so DMA runs are contiguous
    wqkh = nc.dram_tensor("wqkh", [128, CCH * 128], F8, kind="ExternalInput")
    wqkl = nc.dram_tensor("wqkl", [128, CCH * 128], F8, kind="ExternalInput")
    wvh = nc.dram_tensor("wvh", [128, CCH * 64], F8, kind="ExternalInput")
    wvl = nc.dram_tensor("wvl", [128, CCH * 64], F8, kind="ExternalInput")
    ecol = nc.dram_tensor("ecol", [128, 4 * NSLOT], F32, kind="ExternalInput")
    ot = nc.dram_tensor("ot", [H + 1, NQ], F32, kind="ExternalOutput")

    xth_r = xth.rearrange("(n p) t -> p n t", p=128)  # [128, 8, T]
    xtl_r = xtl.rearrange("(n p) t -> p n t", p=128)
    wqkh_r = wqkh.rearrange("p (n m) -> p n m", n=CCH)
    wqkl_r = wqkl.rearrange("p (n m) -> p n m", n=CCH)
    wvh_r = wvh.rearrange("p (n m) -> p n m", n=CCH)
    wvl_r = wvl.rearrange("p (n m) -> p n m", n=CCH)

    # exp engine schedule state: weighted rotation (ACT : DVE : Pool)
    sched = {"n": 0}

    def pick_exp_engine(no_pool=False):
        cyc = ["act", "act", "dve", "act", "act", "dve", "act", "act",
               "dve", "act", "act", "dve", "act", "act", "dve", "act",
               "act", "act"]
        e = cyc[sched["n"] % len(cyc)]
        sched["n"] += 1
        if no_pool and e == "pool":
            e = "act"
        return e

    with tile.TileContext(nc) as tc:
        with (
            tc.tile_pool(name="const", bufs=1) as const_pool,
            tc.tile_pool(name="big", bufs=1) as big_pool,
            tc.tile_pool(name="xin", bufs=3) as xin_pool,
            tc.tile_pool(name="p", bufs=DEPTH + 2) as p_pool,
            tc.tile_pool(name="outp", bufs=2) as out_pool,
            tc.tile_pool(name="ps_st", bufs=DEPTH, space="PSUM") as ps_st,
            tc.tile_pool(name="ps_proj", bufs=2, space="PSUM") as ps_proj,
            tc.tile_pool(name="ps_v", bufs=1, space="PSUM") as ps_v,
            tc.tile_pool(name="ps_o", bufs=1, space="PSUM") as ps_o,
        ):
            # ---- weights first (they gate the first matmul) ----
            F8 = mybir.dt.float8e4
            wqkh_s = const_pool.tile([128, CCH, 128], F8)
            nc.sync.dma_start(wqkh_s[:], wqkh_r)

            xin_tiles = {}

            def load_pair(j, split=False):
                """Start the X^T hi/lo DMAs for permuted positions 2j
                (own block j) and 2j+1 (partner block j) as one 512-wide
                transfer each (512B dram runs keep full DMA rate)."""
                sl = slice(2 * j * QW, (2 * j + 2) * QW)
                xh_t = xin_pool.tile([128, CCH, 2 * QW], F8, tag="xinh")
                xl_t = xin_pool.tile([128, CCH, 2 * QW], F8, tag="xinl")
                if split:
                    h = CCH // 2
                    nc.sync.dma_start(xh_t[:, 0:h], xth_r[:, 0:h, sl])
                    nc.sync.dma_start(xh_t[:, h:CCH], xth_r[:, h:CCH, sl])
                else:
                    nc.sync.dma_start(xh_t[:], xth_r[:, :, sl])
                nc.sync.dma_start(xl_t[:], xtl_r[:, :, sl])
                xin_tiles[j] = (xh_t, xl_t)

            hcch = CCH // 2
            xh0 = xin_pool.tile([128, CCH, 2 * QW], F8, tag="xinh")
            nc.sync.dma_start(xh0[:, 0:hcch], xth_r[:, 0:hcch, 0 : 2 * QW])
            nc.sync.dma_start(xh0[:, hcch:CCH], xth_r[:, hcch:CCH, 0 : 2 * QW])
            wqkl_s = const_pool.tile([128, CCH, 128], F8)
            nc.sync.dma_start(wqkl_s[:], wqkl_r)
            xl0 = xin_pool.tile([128, CCH, 2 * QW], F8, tag="xinl")
            nc.sync.dma_start(xl0[:], xtl_r[:, :, 0 : 2 * QW])
            xin_tiles[0] = (xh0, xl0)
            wvh_s = const_pool.tile([128, CCH, 64], F8)
            nc.sync.dma_start(wvh_s[:], wvh_r)
            wvl_s = const_pool.tile([128, CCH, 64], F8)
            nc.sync.dma_start(wvl_s[:], wvl_r)

            # ---- remaining constants ----
            ecol_s = const_pool.tile([128, 4 * NSLOT], F32)
            nc.sync.dma_start(ecol_s[:], ecol[:])

            # static triangular mask for the diagonal pair:
            # mask01[kp, st, qf] = (qf - 128*st >= kp)
            iota_i = const_pool.tile([128, 2, QW], I32)
            nc.gpsimd.iota(
                iota_i[:], pattern=[[-KT, 2], [1, QW]], base=0,
                channel_multiplier=0,
            )
            iota_ff = const_pool.tile([128, 2, QW], F32)
            nc.vector.tensor_copy(iota_ff[:], iota_i[:])
            kp_i = const_pool.tile([128, 1], I32)
            nc.gpsimd.iota(kp_i[:], pattern=[[0, 1]], base=0,
                           channel_multiplier=1)
            kp_f = const_pool.tile([128, 1], F32)
            nc.vector.tensor_copy(kp_f[:], kp_i[:])
            mask01 = const_pool.tile([128, 2, QW], F16)
            nc.vector.tensor_scalar(
                mask01[:], iota_ff[:], kp_f[:, 0:1], None,
                mybir.AluOpType.is_ge,
            )

            # ---- persistent activations (own|partner permuted order) ----
            q_all = big_pool.tile([64, NQ], F16)  # own queries only
            kT_s = big_pool.tile([64, T], F16)
            v_s = big_pool.tile([128, T // KT, H + 1], F16)
            nc.vector.memset(v_s[:, :, H : H + 1], 1.0)

            proj_chunks = []

            def queue_project_block(pos, last_of_pair):
                """Queue chunked QK+V projection work for permuted
                position pos; chunks are emitted interleaved into the
                attend pair stream so proj matmuls fill the PE while
                exp results are pending."""
                sl = slice(pos * QW, (pos + 1) * QW)
                j, hb = pos // 2, (pos % 2) * QW
                xh_p, xl_p = xin_tiles[j]
                xh_t = xh_p[:, :, hb : hb + QW]
                xl_t = xl_p[:, :, hb : hb + QW]
                if last_of_pair:
                    xin_tiles.pop(j)
                DR = mybir.MatmulPerfMode.DoubleRow
                NP = CCH // 2  # DoubleRow ci-pairs
                passes = [(wqkh_s, xh_t), (wqkl_s, xh_t), (wqkh_s, xl_t)]
                vpasses = [(wvh_s, xh_t), (wvl_s, xh_t), (wvh_s, xl_t)]
                state = {}

                def qk_chunk(pi):
                    def emit():
                        if pi == 0:
                            state["qk"] = ps_proj.tile(
                                [128, QW], F32, tag="qk",
                                name=f"qk_{pos}",
                            )
                        qk_ps = state["qk"]
                        w_s, x_t = passes[pi]
                        for cp in range(NP):
                            nc.tensor.matmul(
                                qk_ps[:],
                                w_s[:, 2 * cp : 2 * cp + 2],
                                x_t[:, 2 * cp : 2 * cp + 2],
                                start=(pi == 0 and cp == 0),
                                stop=(pi == 2 and cp == NP - 1),
                                perf_mode=DR,
                            )
                        if pi == 2:
                            if pos % 2 == 0:  # own block: keep q
                                qsl = slice((pos // 2) * QW,
                                            (pos // 2 + 1) * QW)
                                nc.vector.tensor_copy(
                                    q_all[:, qsl], qk_ps[0:64, :]
                                )
                            nc.vector.tensor_copy(
                                kT_s[:, sl], qk_ps[64:128, :]
                            )
                    return emit

                def v_chunk(g):
                    def emit():
                        if g == 0:
                            state["v"] = ps_v.tile(
                                [128, 2, 64], F32, tag="v",
                                name=f"v_{pos}",
                            )
                        v_ps = state["v"]
                        n = 0
                        for w_s, x_t in vpasses:
                            for cp in range(NP):
                                nc.tensor.matmul(
                                    v_ps[:, g],
                                    x_t[:, 2 * cp : 2 * cp + 2,
                                        g * 128 : (g + 1) * 128],
                                    w_s[:, 2 * cp : 2 * cp + 2],
                                    start=(n == 0),
                                    stop=(n == 3 * NP - 1),
                                    perf_mode=DR,
                                )
                                n += 1
                        if g == 1:
                            nc.vector.tensor_copy(
                                v_s[:, pos * 2 : pos * 2 + 2, 0:H], v_ps[:]
                            )
                    return emit

                for pi in range(3):
                    proj_chunks.append(qk_chunk(pi))
                for g in range(2):
                    proj_chunks.append(v_chunk(g))

            def project_block(pos, last_of_pair):
                queue_project_block(pos, last_of_pair)
                while proj_chunks:
                    proj_chunks.pop(0)()

            def attend_slot(j, reserve=0):
                """Slot j: queries q_all[:, j*256); key pairs: partner
                0..j (filler last), then own 0..j (diagonal last)."""
                q_ap = q_all[0:64, j * QW : (j + 1) * QW]
                o_ps = ps_o.tile([H + 1, QW], F32, tag="o")
                # pair list: (kT col base, kind).  In the final slot the
                # diagonal goes first so the drain tail isn't lengthened
                # by its exp->mask->O chain.
                pairs = [((2 * i + 1) * QW, "filler" if i == j else "plain")
                         for i in range(j + 1)]
                own = [(2 * i * QW, "diag" if i == j else "plain")
                       for i in range(j + 1)]
                if j == NSLOT - 1:
                    own = own[-1:] + own[:-1]
                pairs += own
                npairs = len(pairs)
                st_q = [None] * npairs
                p_q = [None] * npairs

                def emit_s(i):
                    base, _ = pairs[i]
                    st = ps_st.tile([128, 2, QW], F32, tag="st")
                    for s in range(2):
                        nc.tensor.matmul(
                            st[:, s],
                            kT_s[0:64, base + s * KT : base + (s + 1) * KT],
                            q_ap,
                            start=True, stop=True,
                        )
                    st_q[i] = st

                def emit_exp(i):
                    _, kind = pairs[i]
                    st = st_q[i]
                    p_sb = p_pool.tile([128, 2, QW], F16, tag="p")
                    eng = pick_exp_engine(no_pool=(kind == "diag"))
                    if kind == "filler":
                        if eng == "act":
                            nc.scalar.activation(
                                p_sb[:], st[:],
                                mybir.ActivationFunctionType.Exp,
                                scale=ecol_s[:, j : j + 1],
                                bias=ecol_s[:, NSLOT + j : NSLOT + j + 1],
                            )
                        else:
                            e = nc.vector
                            e.tensor_scalar(
                                p_sb.bitcast(I16)[:], st[:],
                                ecol_s[:, 2 * NSLOT + j : 2 * NSLOT + j + 1],
                                ecol_s[:, 3 * NSLOT + j : 3 * NSLOT + j + 1],
                                mybir.AluOpType.mult, mybir.AluOpType.add,
                            )
                    else:
                        if eng == "act":
                            nc.scalar.activation(
                                p_sb[:], st[:],
                                mybir.ActivationFunctionType.Exp,
                                scale=SSC,
                            )
                        else:
                            e = nc.vector
                            e.tensor_scalar(
                                p_sb.bitcast(I16)[:], st[:],
                                FE_A, FE_B,
                                mybir.AluOpType.mult, mybir.AluOpType.add,
                            )
                    if kind == "diag":
                        nc.vector.tensor_tensor(
                            p_sb[:], p_sb[:], mask01[:],
                            mybir.AluOpType.mult,
                        )
                    p_q[i] = p_sb

                def emit_o(i):
                    base, _ = pairs[i]
                    p_sb = p_q[i]
                    for s in range(2):
                        vt = base // KT + s
                        nc.tensor.matmul(
                            o_ps[:], v_s[:, vt, :], p_sb[:, s],
                            start=(i == 0 and s == 0),
                            stop=(i == npairs - 1 and s == 1),
                        )

                # software pipeline: O(i) trails S/exp by DEPTH-1 pairs;
                # queued projection chunks fill the PE in exp windows.
                # `reserve` chunks are held back for the NEXT slot (the
                # final slot has no projection work of its own).
                for i in range(npairs):
                    if i == j and j == NSLOT - 1:
                        # safety: the filler pair reads the last-projected
                        # block; make sure its chunks are all emitted
                        while proj_chunks:
                            proj_chunks.pop(0)()
                    emit_s(i)
                    emit_exp(i)
                    if len(proj_chunks) > reserve:
                        proj_chunks.pop(0)()
                    if i >= DEPTH - 1:
                        emit_o(i - DEPTH + 1)
                while len(proj_chunks) > reserve:
                    proj_chunks.pop(0)()
                for i in range(max(0, npairs - DEPTH + 1), npairs):
                    emit_o(i)

                ot_sb = out_pool.tile([H + 1, QW], F32, tag="ot")
                nc.vector.tensor_copy(ot_sb[:], o_ps[:])
                nc.sync.dma_start(ot[:, j * QW : (j + 1) * QW], ot_sb[:])

            # pipeline: DMA two iterations ahead, projection one ahead, so
            # projection copies enter the Pool/DVE queues before each
            # slot's exp backlog.
            load_pair(1)
            project_block(0, False)
            project_block(1, True)
            for j in range(NSLOT):
                if j + 2 < NSLOT:
                    load_pair(j + 2)
                if j + 1 < NSLOT:
                    queue_project_block(2 * (j + 1), False)
                    queue_project_block(2 * (j + 1) + 1, True)
                attend_slot(j, reserve=5 if j == NSLOT - 2 else 0)

    nc.compile()
    return nc


def _ecol_table(half: int) -> np.ndarray:
    """Per-slot exp-constant columns.  Slot j's filler (partner pair j)
    is visible iff partner global block j precedes own global block j;
    otherwise its exp constants are zeroed so it contributes exact 0.
    Layout: [act_scale(8) | act_bias(8) | fe_a(8) | fe_b(8)]."""
    t = np.zeros((128, 4 * NSLOT), dtype=np.float32)
    own, part = OWN[half], OWN[1 - half]
    for j in range(NSLOT):
        vis = part[j] < own[j]
        t[:, j] = SSC if vis else 0.0
        t[:, NSLOT + j] = 0.0 if vis else -100.0
        t[:, 2 * NSLOT + j] = FE_A if vis else 0.0
        t[:, 3 * NSLOT + j] = FE_B if vis else 0.0
    return t


def kernel(X, Wq, Wk, Wv):
    global _PROGRAM
    X = np.asarray(X, dtype=np.float32)
    Wq = np.asarray(Wq, dtype=np.float32)
    Wk = np.asarray(Wk, dtype=np.float32)
    Wv = np.asarray(Wv, dtype=np.float32)

    if _PROGRAM is None:
        _PROGRAM = _build_program()
    nc = _PROGRAM

    import ml_dtypes
    E4 = ml_dtypes.float8_e4m3fn

    def q8(x):
        return np.asarray(x).astype(E4)

    def swiz(w, m):
        return np.ascontiguousarray(
            w.reshape(CCH, 128, m).transpose(1, 0, 2).reshape(128, -1)
        )

    wqk = np.concatenate([Wq, Wk], axis=1) * WS
    wqk_h = q8(wqk)
    wqk_l = q8(wqk - wqk_h.astype(np.float32))
    wv = Wv * WS
    wv_h = q8(wv)
    wv_l = q8(wv - wv_h.astype(np.float32))
    wqkh_sw, wqkl_sw = swiz(wqk_h, 128), swiz(wqk_l, 128)
    wvh_sw, wvl_sw = swiz(wv_h, 64), swiz(wv_l, 64)

    in_maps = []
    for core in range(NCORES):
        b, half = core // 2, core % 2
        order = [g for p in zip(OWN[half], OWN[1 - half]) for g in p]
        cols = np.concatenate(
            [np.arange(g * QW, (g + 1) * QW) for g in order]
        )
        xp = X[b].T[:, cols]
        xp_h = q8(xp)
        xp_l = q8(xp - xp_h.astype(np.float32))
        in_maps.append(
            {
                "xth": np.ascontiguousarray(xp_h),
                "xtl": np.ascontiguousarray(xp_l),
                "wqkh": wqkh_sw,
                "wqkl": wqkl_sw,
                "wvh": wvh_sw,
                "wvl": wvl_sw,
                "ecol": _ecol_table(half),
            }
        )

    trace = bool(os.environ.get("KERNEL_TRACE"))
    if trace:
        try:
            from antenv.axon_hooks import get_axon_ntff_profile_hook  # noqa: F401
        except ImportError:
            print(
                "KERNEL_TRACE requested but axon NTFF hook unavailable; "
                "running untraced"
            )
            trace = False
    kwargs = {}
    if trace:
        kwargs = dict(
            trace=True,
            trace_cores=[
                int(c)
                for c in os.environ.get("KERNEL_TRACE_CORES", "0").split(",")
            ],
        )
    res = run_bass_kernel_spmd(nc, in_maps, core_ids=list(range(NCORES)), **kwargs)
    if trace:
        print(f"HW exec time: {res.exec_time_ns} ns")
        print(f"mean exec time: {res.mean_exec_time_ns} ns")
        kernel.last_results = res

    out = np.empty((B, T, H), dtype=np.float32)
    for core in range(NCORES):
        b, half = core // 2, core % 2
        oc = res.results[core]["ot"]  # [65, NQ]
        for j, g in enumerate(OWN[half]):
            blk = oc[:, j * QW : (j + 1) * QW]
            out[b, g * QW : (g + 1) * QW] = (blk[0:H] / (blk[H : H + 1] * WS)).T
    return out


# revision 22
# speedup vs baseline: 1.6049x; 1.0122x over previous
"""Causal single-head attention (B=4, T=4096, C=1024, H=64) on 8 TRN2 cores.

Sharding: 2 cores per batch element, causal-balanced interleaved query
blocks of 256: half 0 owns global 256-blocks {0,3,4,7,8,11,12,15}, half 1
owns {1,2,5,6,9,10,13,14}.  With this ownership, a core's slot j (its
j-th owned block, ascending) attends exactly j+1 own blocks and
structurally j+1 partner blocks (one of which may be a fully-masked
filler) — the union is always the causal prefix, and the structural
shape is identical on every core (72 of the ideal 68 [256k x 256q]
tiles vs 80 for a 512-wide split).

The host PERMUTES each core's X^T columns into [own blocks | partner
blocks] order, so the whole device program is static SPMD: slot j's
queries sit at column j*256, its key blocks at [0..j]*256 (own) and
2048 + [0..j]*256 (partner).  Per-core asymmetry lives in:
  - ecol: per-slot exp-constant columns that zero the filler block
    (scale=0 -> exp produces exact 0) when it is acausal,
  - host-side output reassembly (slot -> global block).

The diagonal (own pair j) uses one static triangular mask multiply.

Projections run as 3-pass error-compensated fp8 DoubleRow matmuls
(Xh@Wh + Xh@Wl + Xl@Wh, hi/lo splits prepared on host, weights
pre-scaled x32 so W ~ N(0, C^-1) clears e4m3's subnormal range) at
0.5 cycles/row — 4x the f16 rate.  QK is fused as one [C,128] weight,
H-major; V is projected directly key-major (X^T chunks as the
stationary operand), which halves its PE cost and needs no transposes.
exp is split across ACT (exact) and DVE (one-instruction Schraudolph
fast-exp: int16 <- s*A + B, bitcast to f16, ~1.7% rms, zero mean
bias); Pool/GPSIMD cannot touch PSUM so it only builds constants.
The softmax denominator rides as a ones-column in V;
O^T ([65, 256] numerator+denominator per slot) is DMA'd out raw and the
transpose + division happen on host.  The S->exp->O chain is software-
pipelined (depth 4) so the PE never waits on exp.
"""

import os

import numpy as np

import concourse.bacc as bacc
import concourse.mybir as mybir
import concourse.tile as tile
from concourse.bass_utils import run_bass_kernel_spmd

B, T, C, H = 4, 4096, 1024, 64
NCORES = 8
QW = 256  # query/key block width (ownership granularity)
KT = 128  # k tile (S^T partition dim)
CCH = C // 128  # contraction chunks
NSLOT = 8  # owned query blocks per core
NQ = NSLOT * QW  # queries per core (2048)
HALF = NSLOT * QW  # column offset of partner region (2048)
F32 = mybir.dt.float32
F16 = mybir.dt.float16
I16 = mybir.dt.int16
I32 = mybir.dt.int32

OWN = {
    0: [0, 3, 4, 7, 8, 11, 12, 15],
    1: [1, 2, 5, 6, 9, 10, 13, 14],
}

WS = 32.0  # weight pre-scale so fp8 e4m3 covers W ~ N(0, C^-1)
SSC = float(H) ** -0.5 / (WS * WS)  # score scale: q,k both carry WS
# fast-exp: i16 = int16(s_raw * FE_A + FE_B); bitcast f16 ~= exp(s_raw*SSC)
_LN2 = float(np.log(2.0))
FE_A = 1024.0 / _LN2 * SSC
FE_B = 15.0 * 1024.0 - 59.5

DEPTH = 4  # software pipeline depth of the S->exp->O chain, in pairs

_PROGRAM = None


def _build_program():
    nc = bacc.Bacc(None, target_bir_lowering=False, debug=False)

    F8 = mybir.dt.float8e4
    xth = nc.dram_tensor("xth", [C, T], F8, kind="ExternalInput")
    xtl = nc.dram_tensor("xtl", [C, T], F8, kind="ExternalInput")
    # weights pre-swizzled on host to [p, ci, m] so DMA runs are contiguous
    wqk2 = nc.dram_tensor("wqk2", [128, 2 * CCH * 128], F8,
                          kind="ExternalInput")
    wv2 = nc.dram_tensor("wv2", [128, 2 * CCH * 64], F8,
                         kind="ExternalInput")
    ecol = nc.dram_tensor("ecol", [128, 4 * NSLOT], F32, kind="ExternalInput")
    ot = nc.dram_tensor("ot", [H + 1, NQ], F32, kind="ExternalOutput")

    xth_r = xth.rearrange("(n p) t -> p n t", p=128)  # [128, 8, T]
    xtl_r = xtl.rearrange("(n p) t -> p n t", p=128)
    wqk2_r = wqk2.rearrange("p (h n m) -> p h n m", h=2, n=CCH)
    wv2_r = wv2.rearrange("p (h n m) -> p h n m", h=2, n=CCH)

    # exp engine schedule state: weighted rotation (ACT : DVE : Pool)
    sched = {"n": 0}

    def pick_exp_engine(no_pool=False):
        cyc = ["act", "act", "dve", "act", "act", "dve", "act", "act",
               "dve", "act", "act", "dve", "act", "act", "dve", "act",
               "act", "act"]
        e = cyc[sched["n"] % len(cyc)]
        sched["n"] += 1
        if no_pool and e == "pool":
            e = "act"
        return e

    with tile.TileContext(nc) as tc:
        with (
            tc.tile_pool(name="const", bufs=1) as const_pool,
            tc.tile_pool(name="big", bufs=1) as big_pool,
            tc.tile_pool(name="xin", bufs=3) as xin_pool,
            tc.tile_pool(name="p", bufs=DEPTH + 2) as p_pool,
            tc.tile_pool(name="outp", bufs=2) as out_pool,
            tc.tile_pool(name="ps_st", bufs=DEPTH, space="PSUM") as ps_st,
            tc.tile_pool(name="ps_proj", bufs=2, space="PSUM") as ps_proj,
            tc.tile_pool(name="ps_v", bufs=1, space="PSUM") as ps_v,
            tc.tile_pool(name="ps_o", bufs=1, space="PSUM") as ps_o,
        ):
            # ---- weights first (they gate the first matmul) ----
            F8 = mybir.dt.float8e4
            wqk2_s = const_pool.tile([128, 2, CCH, 128], F8)
            nc.sync.dma_start(wqk2_s[:], wqk2_r)
            wqkh_s, wqkl_s = wqk2_s[:, 0], wqk2_s[:, 1]

            xin_tiles = {}

            def load_pair(j, split=False):
                """Start the X^T hi/lo DMAs for permuted positions 2j
                (own block j) and 2j+1 (partner block j) as one 512-wide
                transfer each (512B dram runs keep full DMA rate)."""
                sl = slice(2 * j * QW, (2 * j + 2) * QW)
                xh_t = xin_pool.tile([128, CCH, 2 * QW], F8, tag="xinh")
                xl_t = xin_pool.tile([128, CCH, 2 * QW], F8, tag="xinl")
                if split:
                    h = CCH // 2
                    nc.sync.dma_start(xh_t[:, 0:h], xth_r[:, 0:h, sl])
                    nc.sync.dma_start(xh_t[:, h:CCH], xth_r[:, h:CCH, sl])
                else:
                    nc.sync.dma_start(xh_t[:], xth_r[:, :, sl])
                nc.sync.dma_start(xl_t[:], xtl_r[:, :, sl])
                xin_tiles[j] = (xh_t, xl_t)

            hcch = CCH // 2
            xh0 = xin_pool.tile([128, CCH, 2 * QW], F8, tag="xinh")
            nc.sync.dma_start(xh0[:, 0:hcch], xth_r[:, 0:hcch, 0 : 2 * QW])
            nc.sync.dma_start(xh0[:, hcch:CCH], xth_r[:, hcch:CCH, 0 : 2 * QW])
            xl0 = xin_pool.tile([128, CCH, 2 * QW], F8, tag="xinl")
            nc.sync.dma_start(xl0[:], xtl_r[:, :, 0 : 2 * QW])
            xin_tiles[0] = (xh0, xl0)
            wv2_s = const_pool.tile([128, 2, CCH, 64], F8)
            nc.sync.dma_start(wv2_s[:], wv2_r)
            wvh_s, wvl_s = wv2_s[:, 0], wv2_s[:, 1]

            # ---- remaining constants ----
            ecol_s = const_pool.tile([128, 4 * NSLOT], F32)
            nc.sync.dma_start(ecol_s[:], ecol[:])

            # static triangular mask for the diagonal pair:
            # mask01[kp, st, qf] = (qf - 128*st >= kp)
            iota_i = const_pool.tile([128, 2, QW], I32)
            nc.gpsimd.iota(
                iota_i[:], pattern=[[-KT, 2], [1, QW]], base=0,
                channel_multiplier=0,
            )
            iota_ff = const_pool.tile([128, 2, QW], F32)
            nc.vector.tensor_copy(iota_ff[:], iota_i[:])
            kp_i = const_pool.tile([128, 1], I32)
            nc.gpsimd.iota(kp_i[:], pattern=[[0, 1]], base=0,
                           channel_multiplier=1)
            kp_f = const_pool.tile([128, 1], F32)
            nc.vector.tensor_copy(kp_f[:], kp_i[:])
            mask01 = const_pool.tile([128, 2, QW], F16)
            nc.vector.tensor_scalar(
                mask01[:], iota_ff[:], kp_f[:, 0:1], None,
                mybir.AluOpType.is_ge,
            )

            # ---- persistent activations (own|partner permuted order) ----
            q_all = big_pool.tile([64, NQ], F16)  # own queries only
            kT_s = big_pool.tile([64, T], F16)
            v_s = big_pool.tile([128, T // KT, H + 1], F16)
            nc.vector.memset(v_s[:, :, H : H + 1], 1.0)

            proj_chunks = []

            def queue_project_block(pos, last_of_pair):
                """Queue chunked QK+V projection work for permuted
                position pos; chunks are emitted interleaved into the
                attend pair stream so proj matmuls fill the PE while
                exp results are pending."""
                sl = slice(pos * QW, (pos + 1) * QW)
                j, hb = pos // 2, (pos % 2) * QW
                xh_p, xl_p = xin_tiles[j]
                xh_t = xh_p[:, :, hb : hb + QW]
                xl_t = xl_p[:, :, hb : hb + QW]
                if last_of_pair:
                    xin_tiles.pop(j)
                DR = mybir.MatmulPerfMode.DoubleRow
                NP = CCH // 2  # DoubleRow ci-pairs
                passes = [(wqkh_s, xh_t), (wqkl_s, xh_t), (wqkh_s, xl_t)]
                vpasses = [(wvh_s, xh_t), (wvl_s, xh_t), (wvh_s, xl_t)]
                state = {}

                def qk_chunk(pi):
                    def emit():
                        if pi == 0:
                            state["qk"] = ps_proj.tile(
                                [128, QW], F32, tag="qk",
                                name=f"qk_{pos}",
                            )
                        qk_ps = state["qk"]
                        w_s, x_t = passes[pi]
                        for cp in range(NP):
                            nc.tensor.matmul(
                                qk_ps[:],
                                w_s[:, 2 * cp : 2 * cp + 2],
                                x_t[:, 2 * cp : 2 * cp + 2],
                                start=(pi == 0 and cp == 0),
                                stop=(pi == 2 and cp == NP - 1),
                                perf_mode=DR,
                            )
                        if pi == 2:
                            if pos % 2 == 0:  # own block: keep q
                                qsl = slice((pos // 2) * QW,
                                            (pos // 2 + 1) * QW)
                                nc.vector.tensor_copy(
                                    q_all[:, qsl], qk_ps[0:64, :]
                                )
                            nc.vector.tensor_copy(
                                kT_s[:, sl], qk_ps[64:128, :]
                            )
                    return emit

                def v_chunk(g):
                    def emit():
                        if g == 0:
                            state["v"] = ps_v.tile(
                                [128, 2, 64], F32, tag="v",
                                name=f"v_{pos}",
                            )
                        v_ps = state["v"]
                        n = 0
                        for w_s, x_t in vpasses:
                            for cp in range(NP):
                                nc.tensor.matmul(
                                    v_ps[:, g],
                                    x_t[:, 2 * cp : 2 * cp + 2,
                                        g * 128 : (g + 1) * 128],
                                    w_s[:, 2 * cp : 2 * cp + 2],
                                    start=(n == 0),
                                    stop=(n == 3 * NP - 1),
                                    perf_mode=DR,
                                )
                                n += 1
                        if g == 1:
                            nc.vector.tensor_copy(
                                v_s[:, pos * 2 : pos * 2 + 2, 0:H], v_ps[:]
                            )
                    return emit

                for pi in range(3):
                    proj_chunks.append(qk_chunk(pi))
                for g in range(2):
                    proj_chunks.append(v_chunk(g))

            def project_block(pos, last_of_pair):
                queue_project_block(pos, last_of_pair)
                while proj_chunks:
                    proj_chunks.pop(0)()

            def attend_slot(j, reserve=0):
                """Slot j: queries q_all[:, j*256); key pairs: partner
                0..j (filler last), then own 0..j (diagonal last)."""
                q_ap = q_all[0:64, j * QW : (j + 1) * QW]
                o_ps = ps_o.tile([H + 1, QW], F32, tag="o")
                # pair list: (kT col base, kind).  In the final slot the
                # diagonal goes first so the drain tail isn't lengthened
                # by its exp->mask->O chain.
                pairs = [((2 * i + 1) * QW, "filler" if i == j else "plain")
                         for i in range(j + 1)]
                own = [(2 * i * QW, "diag" if i == j else "plain")
                       for i in range(j + 1)]
                if j == NSLOT - 1:
                    own = own[-1:] + own[:-1]
                pairs += own
                npairs = len(pairs)
                st_q = [None] * npairs
                p_q = [None] * npairs

                def emit_s(i):
                    base, _ = pairs[i]
                    st = ps_st.tile([128, 2, QW], F32, tag="st")
                    for s in range(2):
                        nc.tensor.matmul(
                            st[:, s],
                            kT_s[0:64, base + s * KT : base + (s + 1) * KT],
                            q_ap,
                            start=True, stop=True,
                        )
                    st_q[i] = st

                def emit_exp(i):
                    _, kind = pairs[i]
                    st = st_q[i]
                    p_sb = p_pool.tile([128, 2, QW], F16, tag="p")
                    if i >= npairs - 7:
                        # tail pairs: strict ACT/DVE alternation (DVE on
                        # the final pair, feeding the DVE o_ps drain
                        # copy) so the bunched trailing O's never wait
                        # on two exps serialized on one engine
                        eng = ("dve" if (npairs - 1 - i) % 2 == 0
                               else "act")
                        sched["n"] += 1
                    else:
                        eng = pick_exp_engine(no_pool=(kind == "diag"))
                    if kind == "filler":
                        if eng == "act":
                            nc.scalar.activation(
                                p_sb[:], st[:],
                                mybir.ActivationFunctionType.Exp,
                                scale=ecol_s[:, j : j + 1],
                                bias=ecol_s[:, NSLOT + j : NSLOT + j + 1],
                            )
                        else:
                            e = nc.vector
                            e.tensor_scalar(
                                p_sb.bitcast(I16)[:], st[:],
                                ecol_s[:, 2 * NSLOT + j : 2 * NSLOT + j + 1],
                                ecol_s[:, 3 * NSLOT + j : 3 * NSLOT + j + 1],
                                mybir.AluOpType.mult, mybir.AluOpType.add,
                            )
                    else:
                        if eng == "act":
                            nc.scalar.activation(
                                p_sb[:], st[:],
                                mybir.ActivationFunctionType.Exp,
                                scale=SSC,
                            )
                        else:
                            e = nc.vector
                            e.tensor_scalar(
                                p_sb.bitcast(I16)[:], st[:],
                                FE_A, FE_B,
                                mybir.AluOpType.mult, mybir.AluOpType.add,
                            )
                    if kind == "diag":
                        nc.vector.tensor_tensor(
                            p_sb[:], p_sb[:], mask01[:],
                            mybir.AluOpType.mult,
                        )
                    p_q[i] = p_sb

                def emit_o(i):
                    base, _ = pairs[i]
                    p_sb = p_q[i]
                    for s in range(2):
                        vt = base // KT + s
                        nc.tensor.matmul(
                            o_ps[:], v_s[:, vt, :], p_sb[:, s],
                            start=(i == 0 and s == 0),
                            stop=(i == npairs - 1 and s == 1),
                        )

                # software pipeline: O(i) trails S/exp by DEPTH-1 pairs;
                # queued projection chunks fill the PE in exp windows.
                # `reserve` chunks are held back for the NEXT slot (the
                # final slot has no projection work of its own).
                for i in range(npairs):
                    if i == j and j == NSLOT - 1:
                        # safety: the filler pair reads the last-projected
                        # block; make sure its chunks are all emitted
                        while proj_chunks:
                            proj_chunks.pop(0)()
                    emit_s(i)
                    emit_exp(i)
                    if len(proj_chunks) > reserve:
                        proj_chunks.pop(0)()
                    if i >= DEPTH - 1:
                        emit_o(i - DEPTH + 1)
                while len(proj_chunks) > reserve:
                    proj_chunks.pop(0)()
                for i in range(max(0, npairs - DEPTH + 1), npairs):
                    emit_o(i)

                ot_sb = out_pool.tile([H + 1, QW], F32, tag="ot")
                nc.vector.tensor_copy(ot_sb[:], o_ps[:])
                nc.sync.dma_start(ot[:, j * QW : (j + 1) * QW], ot_sb[:])

            # pipeline: DMA two iterations ahead, projection one ahead, so
            # projection copies enter the Pool/DVE queues before each
            # slot's exp backlog.
            load_pair(1)
            project_block(0, False)
            project_block(1, True)
            for j in range(NSLOT):
                if j + 2 < NSLOT:
                    load_pair(j + 2)
                if j + 1 < NSLOT:
                    queue_project_block(2 * (j + 1), False)
                    queue_project_block(2 * (j + 1) + 1, True)
                attend_slot(j, reserve={NSLOT - 2: 6, NSLOT - 3: 4}.get(j, 0))

    nc.compile()
    return nc


def _ecol_table(half: int) -> np.ndarray:
    """Per-slot exp-constant columns.  Slot j's filler (partner pair j)
    is visible iff partner global block j precedes own global block j;
    otherwise its exp constants are zeroed so it contributes exact 0.
    Layout: [act_scale(8) | act_bias(8) | fe_a(8) | fe_b(8)]."""
    t = np.zeros((128, 4 * NSLOT), dtype=np.float32)
    own, part = OWN[half], OWN[1 - half]
    for j in range(NSLOT):
        vis = part[j] < own[j]
        t[:, j] = SSC if vis else 0.0
        t[:, NSLOT + j] = 0.0 if vis else -100.0
        t[:, 2 * NSLOT + j] = FE_A if vis else 0.0
        t[:, 3 * NSLOT + j] = FE_B if vis else 0.0
    return t


def kernel(X, Wq, Wk, Wv):
    global _PROGRAM
    X = np.asarray(X, dtype=np.float32)
    Wq = np.asarray(Wq, dtype=np.float32)
    Wk = np.asarray(Wk, dtype=np.float32)
    Wv = np.asarray(Wv, dtype=np.float32)

    if _PROGRAM is None:
        _PROGRAM = _build_program()
    nc = _PROGRAM

    import ml_dtypes
    E4 = ml_dtypes.float8_e4m3fn

    def q8(x):
        return np.asarray(x).astype(E4)

    def swiz(w, m):
        return np.ascontiguousarray(
            w.reshape(CCH, 128, m).transpose(1, 0, 2).reshape(128, -1)
        )

    wqk = np.concatenate([Wq, Wk], axis=1) * WS
    wqk_h = q8(wqk)
    wqk_l = q8(wqk - wqk_h.astype(np.float32))
    wv = Wv * WS
    wv_h = q8(wv)
    wv_l = q8(wv - wv_h.astype(np.float32))
    wqk2_sw = np.ascontiguousarray(
        np.concatenate([swiz(wqk_h, 128), swiz(wqk_l, 128)], axis=1)
    )
    wv2_sw = np.ascontiguousarray(
        np.concatenate([swiz(wv_h, 64), swiz(wv_l, 64)], axis=1)
    )

    in_maps = []
    for core in range(NCORES):
        b, half = core // 2, core % 2
        order = [g for p in zip(OWN[half], OWN[1 - half]) for g in p]
        cols = np.concatenate(
            [np.arange(g * QW, (g + 1) * QW) for g in order]
        )
        xp = X[b].T[:, cols]
        xp_h = q8(xp)
        xp_l = q8(xp - xp_h.astype(np.float32))
        in_maps.append(
            {
                "xth": np.ascontiguousarray(xp_h),
                "xtl": np.ascontiguousarray(xp_l),
                "wqk2": wqk2_sw,
                "wv2": wv2_sw,
                "ecol": _ecol_table(half),
            }
        )

    trace = bool(os.environ.get("KERNEL_TRACE"))
    if trace:
        try:
            from antenv.axon_hooks import get_axon_ntff_profile_hook  # noqa: F401
        except ImportError:
            print(
                "KERNEL_TRACE requested but axon NTFF hook unavailable; "
                "running untraced"
            )
            trace = False
    kwargs = {}
    if trace:
        kwargs = dict(
            trace=True,
            trace_cores=[
                int(c)
                for c in os.environ.get("KERNEL_TRACE_CORES", "0").split(",")
            ],
        )
    res = run_bass_kernel_spmd(nc, in_maps, core_ids=list(range(NCORES)), **kwargs)
    if trace:
        print(f"HW exec time: {res.exec_time_ns} ns")
        print(f"mean exec time: {res.mean_exec_time_ns} ns")
        kernel.last_results = res

    out = np.empty((B, T, H), dtype=np.float32)
    for core in range(NCORES):
        b, half = core // 2, core % 2
        oc = res.results[core]["ot"]  # [65, NQ]
        for j, g in enumerate(OWN[half]):
            blk = oc[:, j * QW : (j + 1) * QW]
            out[b, g * QW : (g + 1) * QW] = (blk[0:H] / (blk[H : H + 1] * WS)).T
    return out


# revision 23
# speedup vs baseline: 1.6185x; 1.0085x over previous
"""Causal single-head attention (B=4, T=4096, C=1024, H=64) on 8 TRN2 cores.

Sharding: 2 cores per batch element, causal-balanced interleaved query
blocks of 256: half 0 owns global 256-blocks {0,3,4,7,8,11,12,15}, half 1
owns {1,2,5,6,9,10,13,14}.  With this ownership, a core's slot j (its
j-th owned block, ascending) attends exactly j+1 own blocks and
structurally j+1 partner blocks (one of which may be a fully-masked
filler) — the union is always the causal prefix, and the structural
shape is identical on every core (72 of the ideal 68 [256k x 256q]
tiles vs 80 for a 512-wide split).

The host PERMUTES each core's X^T columns into [own blocks | partner
blocks] order, so the whole device program is static SPMD: slot j's
queries sit at column j*256, its key blocks at [0..j]*256 (own) and
2048 + [0..j]*256 (partner).  Per-core asymmetry lives in:
  - ecol: per-slot exp-constant columns that zero the filler block
    (scale=0 -> exp produces exact 0) when it is acausal,
  - host-side output reassembly (slot -> global block).

The diagonal (own pair j) uses one static triangular mask multiply.

Projections run as 3-pass error-compensated fp8 DoubleRow matmuls
(Xh@Wh + Xh@Wl + Xl@Wh, hi/lo splits prepared on host, weights
pre-scaled x32 so W ~ N(0, C^-1) clears e4m3's subnormal range) at
0.5 cycles/row — 4x the f16 rate.  QK is fused as one [C,128] weight,
H-major; V is projected directly key-major (X^T chunks as the
stationary operand), which halves its PE cost and needs no transposes.
exp is split across ACT (exact) and DVE (one-instruction Schraudolph
fast-exp: int16 <- s*A + B, bitcast to f16, ~1.7% rms, zero mean
bias); Pool/GPSIMD cannot touch PSUM so it only builds constants.
The softmax denominator rides as a ones-column in V;
O^T ([65, 256] numerator+denominator per slot) is DMA'd out raw and the
transpose + division happen on host.  The S->exp->O chain is software-
pipelined (depth 4) so the PE never waits on exp.
"""

import os

import numpy as np

import concourse.bacc as bacc
import concourse.mybir as mybir
import concourse.tile as tile
from concourse.bass_utils import run_bass_kernel_spmd

B, T, C, H = 4, 4096, 1024, 64
NCORES = 8
QW = 256  # query/key block width (ownership granularity)
KT = 128  # k tile (S^T partition dim)
CCH = C // 128  # contraction chunks
NSLOT = 8  # owned query blocks per core
NQ = NSLOT * QW  # queries per core (2048)
HALF = NSLOT * QW  # column offset of partner region (2048)
F32 = mybir.dt.float32
F16 = mybir.dt.float16
I16 = mybir.dt.int16
I32 = mybir.dt.int32

OWN = {
    0: [0, 3, 4, 7, 8, 11, 12, 15],
    1: [1, 2, 5, 6, 9, 10, 13, 14],
}

WS = 32.0  # weight pre-scale so fp8 e4m3 covers W ~ N(0, C^-1)
SSC = float(H) ** -0.5 / (WS * WS)  # score scale: q,k both carry WS
# fast-exp: i16 = int16(s_raw * FE_A + FE_B); bitcast f16 ~= exp(s_raw*SSC)
_LN2 = float(np.log(2.0))
FE_A = 1024.0 / _LN2 * SSC
FE_B = 15.0 * 1024.0 - 59.5

DEPTH = 4  # software pipeline depth of the S->exp->O chain, in pairs

_PROGRAM = None


def _build_program():
    nc = bacc.Bacc(None, target_bir_lowering=False, debug=False)

    F8 = mybir.dt.float8e4
    xth = nc.dram_tensor("xth", [C, T], F8, kind="ExternalInput")
    xtl = nc.dram_tensor("xtl", [C, T], F8, kind="ExternalInput")
    # weights pre-swizzled on host to [p, ci, m] so DMA runs are contiguous
    wqk2 = nc.dram_tensor("wqk2", [128, 2 * CCH * 128], F8,
                          kind="ExternalInput")
    wv2 = nc.dram_tensor("wv2", [128, 2 * CCH * 64], F8,
                         kind="ExternalInput")
    ecol = nc.dram_tensor("ecol", [128, 4 * NSLOT], F32, kind="ExternalInput")
    ot = nc.dram_tensor("ot", [H + 1, NQ], F32, kind="ExternalOutput")

    xth_r = xth.rearrange("(n p) t -> p n t", p=128)  # [128, 8, T]
    xtl_r = xtl.rearrange("(n p) t -> p n t", p=128)
    wqk2_r = wqk2.rearrange("p (h n m) -> p h n m", h=2, n=CCH)
    wv2_r = wv2.rearrange("p (h n m) -> p h n m", h=2, n=CCH)

    # exp engine schedule state: weighted rotation (ACT : DVE : Pool)
    sched = {"n": 0}

    def pick_exp_engine(no_pool=False):
        cyc = ["act", "act", "dve", "act", "act", "dve", "act", "act",
               "dve", "act", "act", "dve", "act", "act", "dve", "act",
               "act", "act"]
        e = cyc[sched["n"] % len(cyc)]
        sched["n"] += 1
        if no_pool and e == "pool":
            e = "act"
        return e

    with tile.TileContext(nc) as tc:
        with (
            tc.tile_pool(name="const", bufs=1) as const_pool,
            tc.tile_pool(name="big", bufs=1) as big_pool,
            tc.tile_pool(name="xin", bufs=3) as xin_pool,
            tc.tile_pool(name="p", bufs=DEPTH + 2) as p_pool,
            tc.tile_pool(name="outp", bufs=2) as out_pool,
            tc.tile_pool(name="ps_st", bufs=DEPTH, space="PSUM") as ps_st,
            tc.tile_pool(name="ps_proj", bufs=2, space="PSUM") as ps_proj,
            tc.tile_pool(name="ps_v", bufs=1, space="PSUM") as ps_v,
            tc.tile_pool(name="ps_o", bufs=1, space="PSUM") as ps_o,
        ):
            # ---- weights first (they gate the first matmul) ----
            F8 = mybir.dt.float8e4
            wqk2_s = const_pool.tile([128, 2, CCH, 128], F8)
            nc.sync.dma_start(wqk2_s[:], wqk2_r)
            wqkh_s, wqkl_s = wqk2_s[:, 0], wqk2_s[:, 1]

            xin_tiles = {}

            def load_pair(j, split=False):
                """Start the X^T hi/lo DMAs for permuted positions 2j
                (own block j) and 2j+1 (partner block j) as one 512-wide
                transfer each (512B dram runs keep full DMA rate)."""
                sl = slice(2 * j * QW, (2 * j + 2) * QW)
                xh_t = xin_pool.tile([128, CCH, 2 * QW], F8, tag="xinh")
                xl_t = xin_pool.tile([128, CCH, 2 * QW], F8, tag="xinl")
                if split:
                    h = CCH // 2
                    nc.sync.dma_start(xh_t[:, 0:h], xth_r[:, 0:h, sl])
                    nc.sync.dma_start(xh_t[:, h:CCH], xth_r[:, h:CCH, sl])
                else:
                    nc.sync.dma_start(xh_t[:], xth_r[:, :, sl])
                nc.sync.dma_start(xl_t[:], xtl_r[:, :, sl])
                xin_tiles[j] = (xh_t, xl_t)

            hcch = CCH // 2
            xh0 = xin_pool.tile([128, CCH, 2 * QW], F8, tag="xinh")
            nc.sync.dma_start(xh0[:, 0:hcch], xth_r[:, 0:hcch, 0 : 2 * QW])
            nc.sync.dma_start(xh0[:, hcch:CCH], xth_r[:, hcch:CCH, 0 : 2 * QW])
            xl0 = xin_pool.tile([128, CCH, 2 * QW], F8, tag="xinl")
            nc.sync.dma_start(xl0[:], xtl_r[:, :, 0 : 2 * QW])
            xin_tiles[0] = (xh0, xl0)
            wv2_s = const_pool.tile([128, 2, CCH, 64], F8)
            nc.sync.dma_start(wv2_s[:], wv2_r)
            wvh_s, wvl_s = wv2_s[:, 0], wv2_s[:, 1]

            # ---- remaining constants ----
            ecol_s = const_pool.tile([128, 4 * NSLOT], F32)
            nc.sync.dma_start(ecol_s[:], ecol[:])

            # static triangular mask for the diagonal pair:
            # mask01[kp, st, qf] = (qf - 128*st >= kp)
            iota_i = const_pool.tile([128, 2, QW], I32)
            nc.gpsimd.iota(
                iota_i[:], pattern=[[-KT, 2], [1, QW]], base=0,
                channel_multiplier=0,
            )
            iota_ff = const_pool.tile([128, 2, QW], F32)
            nc.vector.tensor_copy(iota_ff[:], iota_i[:])
            kp_i = const_pool.tile([128, 1], I32)
            nc.gpsimd.iota(kp_i[:], pattern=[[0, 1]], base=0,
                           channel_multiplier=1)
            kp_f = const_pool.tile([128, 1], F32)
            nc.vector.tensor_copy(kp_f[:], kp_i[:])
            mask01 = const_pool.tile([128, 2, QW], F16)
            nc.vector.tensor_scalar(
                mask01[:], iota_ff[:], kp_f[:, 0:1], None,
                mybir.AluOpType.is_ge,
            )

            # ---- persistent activations (own|partner permuted order) ----
            q_all = big_pool.tile([64, NQ], F16)  # own queries only
            kT_s = big_pool.tile([64, T], F16)
            v_s = big_pool.tile([128, T // KT, H + 1], F16)
            nc.vector.memset(v_s[:, :, H : H + 1], 1.0)

            proj_chunks = []

            def queue_project_block(pos, last_of_pair):
                """Queue chunked QK+V projection work for permuted
                position pos; chunks are emitted interleaved into the
                attend pair stream so proj matmuls fill the PE while
                exp results are pending."""
                sl = slice(pos * QW, (pos + 1) * QW)
                j, hb = pos // 2, (pos % 2) * QW
                xh_p, xl_p = xin_tiles[j]
                xh_t = xh_p[:, :, hb : hb + QW]
                xl_t = xl_p[:, :, hb : hb + QW]
                if last_of_pair:
                    xin_tiles.pop(j)
                DR = mybir.MatmulPerfMode.DoubleRow
                NP = CCH // 2  # DoubleRow ci-pairs
                passes = [(wqkh_s, xh_t), (wqkl_s, xh_t), (wqkh_s, xl_t)]
                vpasses = [(wvh_s, xh_t), (wvl_s, xh_t), (wvh_s, xl_t)]
                state = {}

                def qk_chunk(pi):
                    def emit():
                        if pi == 0:
                            state["qk"] = ps_proj.tile(
                                [128, QW], F32, tag="qk",
                                name=f"qk_{pos}",
                            )
                        qk_ps = state["qk"]
                        w_s, x_t = passes[pi]
                        for cp in range(NP):
                            nc.tensor.matmul(
                                qk_ps[:],
                                w_s[:, 2 * cp : 2 * cp + 2],
                                x_t[:, 2 * cp : 2 * cp + 2],
                                start=(pi == 0 and cp == 0),
                                stop=(pi == 2 and cp == NP - 1),
                                perf_mode=DR,
                            )
                        if pi == 2:
                            if pos % 2 == 0:  # own block: keep q
                                qsl = slice((pos // 2) * QW,
                                            (pos // 2 + 1) * QW)
                                nc.vector.tensor_copy(
                                    q_all[:, qsl], qk_ps[0:64, :]
                                )
                            nc.vector.tensor_copy(
                                kT_s[:, sl], qk_ps[64:128, :]
                            )
                    return emit

                def v_chunk(g):
                    def emit():
                        if g == 0:
                            state["v"] = ps_v.tile(
                                [128, 2, 64], F32, tag="v",
                                name=f"v_{pos}",
                            )
                        v_ps = state["v"]
                        n = 0
                        for w_s, x_t in vpasses:
                            for cp in range(NP):
                                nc.tensor.matmul(
                                    v_ps[:, g],
                                    x_t[:, 2 * cp : 2 * cp + 2,
                                        g * 128 : (g + 1) * 128],
                                    w_s[:, 2 * cp : 2 * cp + 2],
                                    start=(n == 0),
                                    stop=(n == 3 * NP - 1),
                                    perf_mode=DR,
                                )
                                n += 1
                        if g == 1:
                            nc.scalar.copy(
                                v_s[:, pos * 2 : pos * 2 + 2, 0:H], v_ps[:]
                            )
                    return emit

                for pi in range(3):
                    proj_chunks.append(qk_chunk(pi))
                for g in range(2):
                    proj_chunks.append(v_chunk(g))

            def project_block(pos, last_of_pair):
                queue_project_block(pos, last_of_pair)
                while proj_chunks:
                    proj_chunks.pop(0)()

            def attend_slot(j, reserve=0):
                """Slot j: queries q_all[:, j*256); key pairs: partner
                0..j (filler last), then own 0..j (diagonal last)."""
                q_ap = q_all[0:64, j * QW : (j + 1) * QW]
                o_ps = ps_o.tile([H + 1, QW], F32, tag="o")
                # pair list: (kT col base, kind).  In the final slot the
                # diagonal goes first so the drain tail isn't lengthened
                # by its exp->mask->O chain.
                pairs = [((2 * i + 1) * QW, "filler" if i == j else "plain")
                         for i in range(j + 1)]
                own = [(2 * i * QW, "diag" if i == j else "plain")
                       for i in range(j + 1)]
                if j == NSLOT - 1:
                    own = own[-1:] + own[:-1]
                pairs += own
                npairs = len(pairs)
                st_q = [None] * npairs
                p_q = [None] * npairs

                def emit_s(i):
                    base, _ = pairs[i]
                    st = ps_st.tile([128, 2, QW], F32, tag="st")
                    for s in range(2):
                        nc.tensor.matmul(
                            st[:, s],
                            kT_s[0:64, base + s * KT : base + (s + 1) * KT],
                            q_ap,
                            start=True, stop=True,
                        )
                    st_q[i] = st

                def emit_exp(i):
                    _, kind = pairs[i]
                    st = st_q[i]
                    p_sb = p_pool.tile([128, 2, QW], F16, tag="p")
                    if i >= npairs - 7:
                        # tail pairs: strict ACT/DVE alternation (DVE on
                        # the final pair, feeding the DVE o_ps drain
                        # copy) so the bunched trailing O's never wait
                        # on two exps serialized on one engine
                        eng = ("dve" if (npairs - 1 - i) % 2 == 0
                               else "act")
                        sched["n"] += 1
                    else:
                        eng = pick_exp_engine(no_pool=(kind == "diag"))
                    if kind == "filler":
                        if eng == "act":
                            nc.scalar.activation(
                                p_sb[:], st[:],
                                mybir.ActivationFunctionType.Exp,
                                scale=ecol_s[:, j : j + 1],
                                bias=ecol_s[:, NSLOT + j : NSLOT + j + 1],
                            )
                        else:
                            e = nc.vector
                            e.tensor_scalar(
                                p_sb.bitcast(I16)[:], st[:],
                                ecol_s[:, 2 * NSLOT + j : 2 * NSLOT + j + 1],
                                ecol_s[:, 3 * NSLOT + j : 3 * NSLOT + j + 1],
                                mybir.AluOpType.mult, mybir.AluOpType.add,
                            )
                    else:
                        if eng == "act":
                            nc.scalar.activation(
                                p_sb[:], st[:],
                                mybir.ActivationFunctionType.Exp,
                                scale=SSC,
                            )
                        else:
                            e = nc.vector
                            e.tensor_scalar(
                                p_sb.bitcast(I16)[:], st[:],
                                FE_A, FE_B,
                                mybir.AluOpType.mult, mybir.AluOpType.add,
                            )
                    if kind == "diag":
                        nc.vector.tensor_tensor(
                            p_sb[:], p_sb[:], mask01[:],
                            mybir.AluOpType.mult,
                        )
                    p_q[i] = p_sb

                def emit_o(i):
                    base, _ = pairs[i]
                    p_sb = p_q[i]
                    for s in range(2):
                        vt = base // KT + s
                        nc.tensor.matmul(
                            o_ps[:], v_s[:, vt, :], p_sb[:, s],
                            start=(i == 0 and s == 0),
                            stop=(i == npairs - 1 and s == 1),
                        )

                # software pipeline: O(i) trails S/exp by DEPTH-1 pairs;
                # queued projection chunks fill the PE in exp windows.
                # `reserve` chunks are held back for the NEXT slot (the
                # final slot has no projection work of its own).
                for i in range(npairs):
                    if i == j and j == NSLOT - 1:
                        # safety: the filler pair reads the last-projected
                        # block; make sure its chunks are all emitted
                        while proj_chunks:
                            proj_chunks.pop(0)()
                    emit_s(i)
                    emit_exp(i)
                    if len(proj_chunks) > reserve:
                        proj_chunks.pop(0)()
                    if i >= DEPTH - 1:
                        emit_o(i - DEPTH + 1)
                while len(proj_chunks) > reserve:
                    proj_chunks.pop(0)()
                for i in range(max(0, npairs - DEPTH + 1), npairs):
                    emit_o(i)

                ot_sb = out_pool.tile([H + 1, QW], F32, tag="ot")
                nc.vector.tensor_copy(ot_sb[:], o_ps[:])
                nc.sync.dma_start(ot[:, j * QW : (j + 1) * QW], ot_sb[:])

            # pipeline: DMA two iterations ahead, projection one ahead, so
            # projection copies enter the Pool/DVE queues before each
            # slot's exp backlog.
            load_pair(1)
            project_block(0, False)
            project_block(1, True)
            for j in range(NSLOT):
                if j + 2 < NSLOT:
                    load_pair(j + 2)
                if j + 1 < NSLOT:
                    queue_project_block(2 * (j + 1), False)
                    queue_project_block(2 * (j + 1) + 1, True)
                attend_slot(j, reserve={NSLOT - 2: 6, NSLOT - 3: 4}.get(j, 0))

    nc.compile()
    return nc


def _ecol_table(half: int) -> np.ndarray:
    """Per-slot exp-constant columns.  Slot j's filler (partner pair j)
    is visible iff partner global block j precedes own global block j;
    otherwise its exp constants are zeroed so it contributes exact 0.
    Layout: [act_scale(8) | act_bias(8) | fe_a(8) | fe_b(8)]."""
    t = np.zeros((128, 4 * NSLOT), dtype=np.float32)
    own, part = OWN[half], OWN[1 - half]
    for j in range(NSLOT):
        vis = part[j] < own[j]
        t[:, j] = SSC if vis else 0.0
        t[:, NSLOT + j] = 0.0 if vis else -100.0
        t[:, 2 * NSLOT + j] = FE_A if vis else 0.0
        t[:, 3 * NSLOT + j] = FE_B if vis else 0.0
    return t


def kernel(X, Wq, Wk, Wv):
    global _PROGRAM
    X = np.asarray(X, dtype=np.float32)
    Wq = np.asarray(Wq, dtype=np.float32)
    Wk = np.asarray(Wk, dtype=np.float32)
    Wv = np.asarray(Wv, dtype=np.float32)

    if _PROGRAM is None:
        _PROGRAM = _build_program()
    nc = _PROGRAM

    import ml_dtypes
    E4 = ml_dtypes.float8_e4m3fn

    def q8(x):
        return np.asarray(x).astype(E4)

    def swiz(w, m):
        return np.ascontiguousarray(
            w.reshape(CCH, 128, m).transpose(1, 0, 2).reshape(128, -1)
        )

    wqk = np.concatenate([Wq, Wk], axis=1) * WS
    wqk_h = q8(wqk)
    wqk_l = q8(wqk - wqk_h.astype(np.float32))
    wv = Wv * WS
    wv_h = q8(wv)
    wv_l = q8(wv - wv_h.astype(np.float32))
    wqk2_sw = np.ascontiguousarray(
        np.concatenate([swiz(wqk_h, 128), swiz(wqk_l, 128)], axis=1)
    )
    wv2_sw = np.ascontiguousarray(
        np.concatenate([swiz(wv_h, 64), swiz(wv_l, 64)], axis=1)
    )

    in_maps = []
    for core in range(NCORES):
        b, half = core // 2, core % 2
        order = [g for p in zip(OWN[half], OWN[1 - half]) for g in p]
        cols = np.concatenate(
            [np.arange(g * QW, (g + 1) * QW) for g in order]
        )
        xp = X[b].T[:, cols]
        xp_h = q8(xp)
        xp_l = q8(xp - xp_h.astype(np.float32))
        in_maps.append(
            {
                "xth": np.ascontiguousarray(xp_h),
                "xtl": np.ascontiguousarray(xp_l),
                "wqk2": wqk2_sw,
                "wv2": wv2_sw,
                "ecol": _ecol_table(half),
            }
        )

    trace = bool(os.environ.get("KERNEL_TRACE"))
    if trace:
        try:
            from antenv.axon_hooks import get_axon_ntff_profile_hook  # noqa: F401
        except ImportError:
            print(
                "KERNEL_TRACE requested but axon NTFF hook unavailable; "
                "running untraced"
            )
            trace = False
    kwargs = {}
    if trace:
        kwargs = dict(
            trace=True,
            trace_cores=[
                int(c)
                for c in os.environ.get("KERNEL_TRACE_CORES", "0").split(",")
            ],
        )
    res = run_bass_kernel_spmd(nc, in_maps, core_ids=list(range(NCORES)), **kwargs)
    if trace:
        print(f"HW exec time: {res.exec_time_ns} ns")
        print(f"mean exec time: {res.mean_exec_time_ns} ns")
        kernel.last_results = res

    out = np.empty((B, T, H), dtype=np.float32)
    for core in range(NCORES):
        b, half = core // 2, core % 2
        oc = res.results[core]["ot"]  # [65, NQ]
        for j, g in enumerate(OWN[half]):
            blk = oc[:, j * QW : (j + 1) * QW]
            out[b, g * QW : (g + 1) * QW] = (blk[0:H] / (blk[H : H + 1] * WS)).T
    return out


# revision 24
# speedup vs baseline: 1.6216x; 1.0019x over previous
"""Causal single-head attention (B=4, T=4096, C=1024, H=64) on 8 TRN2 cores.

Sharding: 2 cores per batch element, causal-balanced interleaved query
blocks of 256: half 0 owns global 256-blocks {0,3,4,7,8,11,12,15}, half 1
owns {1,2,5,6,9,10,13,14}.  With this ownership, a core's slot j (its
j-th owned block, ascending) attends exactly j+1 own blocks and
structurally j+1 partner blocks (one of which may be a fully-masked
filler) — the union is always the causal prefix, and the structural
shape is identical on every core (72 of the ideal 68 [256k x 256q]
tiles vs 80 for a 512-wide split).

The host PERMUTES each core's X^T columns into [own blocks | partner
blocks] order, so the whole device program is static SPMD: slot j's
queries sit at column j*256, its key blocks at [0..j]*256 (own) and
2048 + [0..j]*256 (partner).  Per-core asymmetry lives in:
  - ecol: per-slot exp-constant columns that zero the filler block
    (scale=0 -> exp produces exact 0) when it is acausal,
  - host-side output reassembly (slot -> global block).

The diagonal (own pair j) uses one static triangular mask multiply.

Projections run as 3-pass error-compensated fp8 DoubleRow matmuls
(Xh@Wh + Xh@Wl + Xl@Wh, hi/lo splits prepared on host, weights
pre-scaled x32 so W ~ N(0, C^-1) clears e4m3's subnormal range) at
0.5 cycles/row — 4x the f16 rate.  QK is fused as one [C,128] weight,
H-major; V is projected directly key-major (X^T chunks as the
stationary operand), which halves its PE cost and needs no transposes.
exp is split across ACT (exact) and DVE (one-instruction Schraudolph
fast-exp: int16 <- s*A + B, bitcast to f16, ~1.7% rms, zero mean
bias); Pool/GPSIMD cannot touch PSUM so it only builds constants.
The softmax denominator rides as a ones-column in V;
O^T ([65, 256] numerator+denominator per slot) is DMA'd out raw and the
transpose + division happen on host.  The S->exp->O chain is software-
pipelined (depth 4) so the PE never waits on exp.
"""

import os

import numpy as np

import concourse.bacc as bacc
import concourse.mybir as mybir
import concourse.tile as tile
from concourse.bass_utils import run_bass_kernel_spmd

B, T, C, H = 4, 4096, 1024, 64
NCORES = 8
QW = 256  # query/key block width (ownership granularity)
KT = 128  # k tile (S^T partition dim)
CCH = C // 128  # contraction chunks
NSLOT = 8  # owned query blocks per core
NQ = NSLOT * QW  # queries per core (2048)
HALF = NSLOT * QW  # column offset of partner region (2048)
F32 = mybir.dt.float32
F16 = mybir.dt.float16
I16 = mybir.dt.int16
I32 = mybir.dt.int32

OWN = {
    0: [0, 3, 4, 7, 8, 11, 12, 15],
    1: [1, 2, 5, 6, 9, 10, 13, 14],
}

WS = 32.0  # weight pre-scale so fp8 e4m3 covers W ~ N(0, C^-1)
SSC = float(H) ** -0.5 / (WS * WS)  # score scale: q,k both carry WS
# fast-exp: i16 = int16(s_raw * FE_A + FE_B); bitcast f16 ~= exp(s_raw*SSC)
_LN2 = float(np.log(2.0))
FE_A = 1024.0 / _LN2 * SSC
FE_B = 15.0 * 1024.0 - 59.5

DEPTH = 4  # software pipeline depth of the S->exp->O chain, in pairs

_PROGRAM = None


def _build_program():
    nc = bacc.Bacc(None, target_bir_lowering=False, debug=False)

    F8 = mybir.dt.float8e4
    xth = nc.dram_tensor("xth", [C, T], F8, kind="ExternalInput")
    xtl = nc.dram_tensor("xtl", [C, T], F8, kind="ExternalInput")
    # weights pre-swizzled on host to [p, ci, m] so DMA runs are contiguous
    wqk2 = nc.dram_tensor("wqk2", [128, 2 * CCH * 128], F8,
                          kind="ExternalInput")
    wv2 = nc.dram_tensor("wv2", [128, 2 * CCH * 64], F8,
                         kind="ExternalInput")
    ecol = nc.dram_tensor("ecol", [128, 4 * NSLOT], F32, kind="ExternalInput")
    ot = nc.dram_tensor("ot", [H + 1, NQ], F32, kind="ExternalOutput")

    xth_r = xth.rearrange("(n p) t -> p n t", p=128)  # [128, 8, T]
    xtl_r = xtl.rearrange("(n p) t -> p n t", p=128)
    wqk2_r = wqk2.rearrange("p (h n m) -> p h n m", h=2, n=CCH)
    wv2_r = wv2.rearrange("p (h n m) -> p h n m", h=2, n=CCH)

    # exp engine schedule state: weighted rotation (ACT : DVE : Pool)
    sched = {"n": 0}

    def pick_exp_engine(no_pool=False):
        cyc = ["act", "act", "dve", "act", "act", "dve", "act", "act",
               "dve", "act", "act", "dve", "act", "act", "dve", "act",
               "act", "act"]
        e = cyc[sched["n"] % len(cyc)]
        sched["n"] += 1
        if no_pool and e == "pool":
            e = "act"
        return e

    with tile.TileContext(nc) as tc:
        with (
            tc.tile_pool(name="const", bufs=1) as const_pool,
            tc.tile_pool(name="big", bufs=1) as big_pool,
            tc.tile_pool(name="xin", bufs=3) as xin_pool,
            tc.tile_pool(name="p", bufs=DEPTH + 2) as p_pool,
            tc.tile_pool(name="outp", bufs=2) as out_pool,
            tc.tile_pool(name="ps_st", bufs=DEPTH, space="PSUM") as ps_st,
            tc.tile_pool(name="ps_proj", bufs=2, space="PSUM") as ps_proj,
            tc.tile_pool(name="ps_v", bufs=1, space="PSUM") as ps_v,
            tc.tile_pool(name="ps_o", bufs=1, space="PSUM") as ps_o,
        ):
            # ---- weights first (they gate the first matmul) ----
            F8 = mybir.dt.float8e4
            wqk2_s = const_pool.tile([128, 2, CCH, 128], F8)
            nc.sync.dma_start(wqk2_s[:], wqk2_r)
            wqkh_s, wqkl_s = wqk2_s[:, 0], wqk2_s[:, 1]

            xin_tiles = {}

            def load_pair(j, split=False):
                """Start the X^T hi/lo DMAs for permuted positions 2j
                (own block j) and 2j+1 (partner block j) as one 512-wide
                transfer each (512B dram runs keep full DMA rate)."""
                sl = slice(2 * j * QW, (2 * j + 2) * QW)
                xh_t = xin_pool.tile([128, CCH, 2 * QW], F8, tag="xinh")
                xl_t = xin_pool.tile([128, CCH, 2 * QW], F8, tag="xinl")
                if split:
                    h = CCH // 2
                    nc.sync.dma_start(xh_t[:, 0:h], xth_r[:, 0:h, sl])
                    nc.sync.dma_start(xh_t[:, h:CCH], xth_r[:, h:CCH, sl])
                else:
                    nc.sync.dma_start(xh_t[:], xth_r[:, :, sl])
                nc.sync.dma_start(xl_t[:], xtl_r[:, :, sl])
                xin_tiles[j] = (xh_t, xl_t)

            hcch = CCH // 2
            xh0 = xin_pool.tile([128, CCH, 2 * QW], F8, tag="xinh")
            nc.sync.dma_start(xh0[:, 0:hcch], xth_r[:, 0:hcch, 0 : 2 * QW])
            nc.sync.dma_start(xh0[:, hcch:CCH], xth_r[:, hcch:CCH, 0 : 2 * QW])
            xl0 = xin_pool.tile([128, CCH, 2 * QW], F8, tag="xinl")
            nc.sync.dma_start(xl0[:], xtl_r[:, :, 0 : 2 * QW])
            xin_tiles[0] = (xh0, xl0)
            wv2_s = const_pool.tile([128, 2, CCH, 64], F8)
            nc.sync.dma_start(wv2_s[:], wv2_r)
            wvh_s, wvl_s = wv2_s[:, 0], wv2_s[:, 1]

            # ---- remaining constants ----
            ecol_s = const_pool.tile([128, 4 * NSLOT], F32)
            nc.sync.dma_start(ecol_s[:], ecol[:])

            # static triangular mask for the diagonal pair:
            # mask01[kp, st, qf] = (qf - 128*st >= kp)
            iota_i = const_pool.tile([128, 2, QW], I32)
            nc.gpsimd.iota(
                iota_i[:], pattern=[[-KT, 2], [1, QW]], base=0,
                channel_multiplier=0,
            )
            iota_ff = const_pool.tile([128, 2, QW], F32)
            nc.vector.tensor_copy(iota_ff[:], iota_i[:])
            kp_i = const_pool.tile([128, 1], I32)
            nc.gpsimd.iota(kp_i[:], pattern=[[0, 1]], base=0,
                           channel_multiplier=1)
            kp_f = const_pool.tile([128, 1], F32)
            nc.vector.tensor_copy(kp_f[:], kp_i[:])
            mask01 = const_pool.tile([128, 2, QW], F16)
            nc.vector.tensor_scalar(
                mask01[:], iota_ff[:], kp_f[:, 0:1], None,
                mybir.AluOpType.is_ge,
            )

            # ---- persistent activations (own|partner permuted order) ----
            q_all = big_pool.tile([64, NQ], F16)  # own queries only
            kT_s = big_pool.tile([64, T], F16)
            v_s = big_pool.tile([128, T // KT, H + 1], F16)
            nc.vector.memset(v_s[:, :, H : H + 1], 1.0)

            proj_chunks = []

            def queue_project_block(pos, last_of_pair):
                """Queue chunked QK+V projection work for permuted
                position pos; chunks are emitted interleaved into the
                attend pair stream so proj matmuls fill the PE while
                exp results are pending."""
                sl = slice(pos * QW, (pos + 1) * QW)
                j, hb = pos // 2, (pos % 2) * QW
                xh_p, xl_p = xin_tiles[j]
                xh_t = xh_p[:, :, hb : hb + QW]
                xl_t = xl_p[:, :, hb : hb + QW]
                if last_of_pair:
                    xin_tiles.pop(j)
                DR = mybir.MatmulPerfMode.DoubleRow
                NP = CCH // 2  # DoubleRow ci-pairs
                passes = [(wqkh_s, xh_t), (wqkl_s, xh_t), (wqkh_s, xl_t)]
                vpasses = [(wvh_s, xh_t), (wvl_s, xh_t), (wvh_s, xl_t)]
                state = {}

                def qk_chunk(pi):
                    def emit():
                        if pi == 0:
                            state["qk"] = ps_proj.tile(
                                [128, QW], F32, tag="qk",
                                name=f"qk_{pos}",
                            )
                        qk_ps = state["qk"]
                        w_s, x_t = passes[pi]
                        for cp in range(NP):
                            nc.tensor.matmul(
                                qk_ps[:],
                                w_s[:, 2 * cp : 2 * cp + 2],
                                x_t[:, 2 * cp : 2 * cp + 2],
                                start=(pi == 0 and cp == 0),
                                stop=(pi == 2 and cp == NP - 1),
                                perf_mode=DR,
                            )
                        if pi == 2:
                            if pos % 2 == 0:  # own block: keep q
                                qsl = slice((pos // 2) * QW,
                                            (pos // 2 + 1) * QW)
                                nc.scalar.copy(
                                    q_all[:, qsl], qk_ps[0:64, :]
                                )
                            nc.vector.tensor_copy(
                                kT_s[:, sl], qk_ps[64:128, :]
                            )
                    return emit

                def v_chunk(g):
                    def emit():
                        if g == 0:
                            state["v"] = ps_v.tile(
                                [128, 2, 64], F32, tag="v",
                                name=f"v_{pos}",
                            )
                        v_ps = state["v"]
                        n = 0
                        for w_s, x_t in vpasses:
                            for cp in range(NP):
                                nc.tensor.matmul(
                                    v_ps[:, g],
                                    x_t[:, 2 * cp : 2 * cp + 2,
                                        g * 128 : (g + 1) * 128],
                                    w_s[:, 2 * cp : 2 * cp + 2],
                                    start=(n == 0),
                                    stop=(n == 3 * NP - 1),
                                    perf_mode=DR,
                                )
                                n += 1
                        if g == 1:
                            nc.scalar.copy(
                                v_s[:, pos * 2 : pos * 2 + 2, 0:H], v_ps[:]
                            )
                    return emit

                for pi in range(3):
                    proj_chunks.append(qk_chunk(pi))
                for g in range(2):
                    proj_chunks.append(v_chunk(g))

            def project_block(pos, last_of_pair):
                queue_project_block(pos, last_of_pair)
                while proj_chunks:
                    proj_chunks.pop(0)()

            def attend_slot(j, reserve=0):
                """Slot j: queries q_all[:, j*256); key pairs: partner
                0..j (filler last), then own 0..j (diagonal last)."""
                q_ap = q_all[0:64, j * QW : (j + 1) * QW]
                o_ps = ps_o.tile([H + 1, QW], F32, tag="o")
                # pair list: (kT col base, kind).  In the final slot the
                # diagonal goes first so the drain tail isn't lengthened
                # by its exp->mask->O chain.
                pairs = [((2 * i + 1) * QW, "filler" if i == j else "plain")
                         for i in range(j + 1)]
                own = [(2 * i * QW, "diag" if i == j else "plain")
                       for i in range(j + 1)]
                if j == NSLOT - 1:
                    own = own[-1:] + own[:-1]
                pairs += own
                npairs = len(pairs)
                st_q = [None] * npairs
                p_q = [None] * npairs

                def emit_s(i):
                    base, _ = pairs[i]
                    st = ps_st.tile([128, 2, QW], F32, tag="st")
                    for s in range(2):
                        nc.tensor.matmul(
                            st[:, s],
                            kT_s[0:64, base + s * KT : base + (s + 1) * KT],
                            q_ap,
                            start=True, stop=True,
                        )
                    st_q[i] = st

                def emit_exp(i):
                    _, kind = pairs[i]
                    st = st_q[i]
                    p_sb = p_pool.tile([128, 2, QW], F16, tag="p")
                    if i >= npairs - 7:
                        # tail pairs: strict ACT/DVE alternation (DVE on
                        # the final pair, feeding the DVE o_ps drain
                        # copy) so the bunched trailing O's never wait
                        # on two exps serialized on one engine
                        eng = ("dve" if (npairs - 1 - i) % 2 == 0
                               else "act")
                        sched["n"] += 1
                    else:
                        eng = pick_exp_engine(no_pool=(kind == "diag"))
                    if kind == "filler":
                        if eng == "act":
                            nc.scalar.activation(
                                p_sb[:], st[:],
                                mybir.ActivationFunctionType.Exp,
                                scale=ecol_s[:, j : j + 1],
                                bias=ecol_s[:, NSLOT + j : NSLOT + j + 1],
                            )
                        else:
                            e = nc.vector
                            e.tensor_scalar(
                                p_sb.bitcast(I16)[:], st[:],
                                ecol_s[:, 2 * NSLOT + j : 2 * NSLOT + j + 1],
                                ecol_s[:, 3 * NSLOT + j : 3 * NSLOT + j + 1],
                                mybir.AluOpType.mult, mybir.AluOpType.add,
                            )
                    else:
                        if eng == "act":
                            nc.scalar.activation(
                                p_sb[:], st[:],
                                mybir.ActivationFunctionType.Exp,
                                scale=SSC,
                            )
                        else:
                            e = nc.vector
                            e.tensor_scalar(
                                p_sb.bitcast(I16)[:], st[:],
                                FE_A, FE_B,
                                mybir.AluOpType.mult, mybir.AluOpType.add,
                            )
                    if kind == "diag":
                        nc.vector.tensor_tensor(
                            p_sb[:], p_sb[:], mask01[:],
                            mybir.AluOpType.mult,
                        )
                    p_q[i] = p_sb

                def emit_o(i):
                    base, _ = pairs[i]
                    p_sb = p_q[i]
                    for s in range(2):
                        vt = base // KT + s
                        nc.tensor.matmul(
                            o_ps[:], v_s[:, vt, :], p_sb[:, s],
                            start=(i == 0 and s == 0),
                            stop=(i == npairs - 1 and s == 1),
                        )

                # software pipeline: O(i) trails S/exp by DEPTH-1 pairs;
                # queued projection chunks fill the PE in exp windows.
                # `reserve` chunks are held back for the NEXT slot (the
                # final slot has no projection work of its own).
                for i in range(npairs):
                    if i == j and j == NSLOT - 1:
                        # safety: the filler pair reads the last-projected
                        # block; make sure its chunks are all emitted
                        while proj_chunks:
                            proj_chunks.pop(0)()
                    emit_s(i)
                    emit_exp(i)
                    if len(proj_chunks) > reserve:
                        proj_chunks.pop(0)()
                    if i >= DEPTH - 1:
                        emit_o(i - DEPTH + 1)
                while len(proj_chunks) > reserve:
                    proj_chunks.pop(0)()
                for i in range(max(0, npairs - DEPTH + 1), npairs):
                    emit_o(i)

                ot_sb = out_pool.tile([H + 1, QW], F32, tag="ot")
                nc.vector.tensor_copy(ot_sb[:], o_ps[:])
                nc.sync.dma_start(ot[:, j * QW : (j + 1) * QW], ot_sb[:])

            # pipeline: DMA two iterations ahead, projection one ahead, so
            # projection copies enter the Pool/DVE queues before each
            # slot's exp backlog.
            load_pair(1)
            project_block(0, False)
            project_block(1, True)
            for j in range(NSLOT):
                if j + 2 < NSLOT:
                    load_pair(j + 2)
                if j + 1 < NSLOT:
                    queue_project_block(2 * (j + 1), False)
                    queue_project_block(2 * (j + 1) + 1, True)
                attend_slot(j, reserve={NSLOT - 2: 6, NSLOT - 3: 4}.get(j, 0))

    nc.compile()
    return nc


def _ecol_table(half: int) -> np.ndarray:
    """Per-slot exp-constant columns.  Slot j's filler (partner pair j)
    is visible iff partner global block j precedes own global block j;
    otherwise its exp constants are zeroed so it contributes exact 0.
    Layout: [act_scale(8) | act_bias(8) | fe_a(8) | fe_b(8)]."""
    t = np.zeros((128, 4 * NSLOT), dtype=np.float32)
    own, part = OWN[half], OWN[1 - half]
    for j in range(NSLOT):
        vis = part[j] < own[j]
        t[:, j] = SSC if vis else 0.0
        t[:, NSLOT + j] = 0.0 if vis else -100.0
        t[:, 2 * NSLOT + j] = FE_A if vis else 0.0
        t[:, 3 * NSLOT + j] = FE_B if vis else 0.0
    return t


def kernel(X, Wq, Wk, Wv):
    global _PROGRAM
    X = np.asarray(X, dtype=np.float32)
    Wq = np.asarray(Wq, dtype=np.float32)
    Wk = np.asarray(Wk, dtype=np.float32)
    Wv = np.asarray(Wv, dtype=np.float32)

    if _PROGRAM is None:
        _PROGRAM = _build_program()
    nc = _PROGRAM

    import ml_dtypes
    E4 = ml_dtypes.float8_e4m3fn

    def q8(x):
        return np.asarray(x).astype(E4)

    def swiz(w, m):
        return np.ascontiguousarray(
            w.reshape(CCH, 128, m).transpose(1, 0, 2).reshape(128, -1)
        )

    wqk = np.concatenate([Wq, Wk], axis=1) * WS
    wqk_h = q8(wqk)
    wqk_l = q8(wqk - wqk_h.astype(np.float32))
    wv = Wv * WS
    wv_h = q8(wv)
    wv_l = q8(wv - wv_h.astype(np.float32))
    wqk2_sw = np.ascontiguousarray(
        np.concatenate([swiz(wqk_h, 128), swiz(wqk_l, 128)], axis=1)
    )
    wv2_sw = np.ascontiguousarray(
        np.concatenate([swiz(wv_h, 64), swiz(wv_l, 64)], axis=1)
    )

    in_maps = []
    for core in range(NCORES):
        b, half = core // 2, core % 2
        order = [g for p in zip(OWN[half], OWN[1 - half]) for g in p]
        cols = np.concatenate(
            [np.arange(g * QW, (g + 1) * QW) for g in order]
        )
        xp = X[b].T[:, cols]
        xp_h = q8(xp)
        xp_l = q8(xp - xp_h.astype(np.float32))
        in_maps.append(
            {
                "xth": np.ascontiguousarray(xp_h),
                "xtl": np.ascontiguousarray(xp_l),
                "wqk2": wqk2_sw,
                "wv2": wv2_sw,
                "ecol": _ecol_table(half),
            }
        )

    trace = bool(os.environ.get("KERNEL_TRACE"))
    if trace:
        try:
            from antenv.axon_hooks import get_axon_ntff_profile_hook  # noqa: F401
        except ImportError:
            print(
                "KERNEL_TRACE requested but axon NTFF hook unavailable; "
                "running untraced"
            )
            trace = False
    kwargs = {}
    if trace:
        kwargs = dict(
            trace=True,
            trace_cores=[
                int(c)
                for c in os.environ.get("KERNEL_TRACE_CORES", "0").split(",")
            ],
        )
    res = run_bass_kernel_spmd(nc, in_maps, core_ids=list(range(NCORES)), **kwargs)
    if trace:
        print(f"HW exec time: {res.exec_time_ns} ns")
        print(f"mean exec time: {res.mean_exec_time_ns} ns")
        kernel.last_results = res

    out = np.empty((B, T, H), dtype=np.float32)
    for core in range(NCORES):
        b, half = core // 2, core % 2
        oc = res.results[core]["ot"]  # [65, NQ]
        for j, g in enumerate(OWN[half]):
            blk = oc[:, j * QW : (j + 1) * QW]
            out[b, g * QW : (g + 1) * QW] = (blk[0:H] / (blk[H : H + 1] * WS)).T
    return out
